# revision 1
# baseline (speedup 1.0000x reference)
"""Trainium2 Bass kernel for BipartiteSAGE-style 2-layer GraphConv.

Reference computation (N=120000 nodes, E=1e6 edges, EMB=128, HID=256, OUT=128):
    pol = relu(pol_features @ W_proj + b_proj) + state_emb[state_ids]   [100000,128]
    x   = concat([pol, emb_tick])                                        [N,128]
    agg = segment_sum(x[src]*w, dst);  h = relu(agg@W1_rel + b1 + x@W1_root)
    agg2= segment_sum(h[src]*w, dst);  out = agg2@W2_rel + b2 + h@W2_root

Distribution: 8 NeuronCores, edges sharded by DESTINATION range (each core owns
15000 nodes). Per-edge aggregation is done as PE matmuls: for each 128-edge
block, a host-built [128 edges, 128 dst-slot] weight matrix A (bf16, edge
weight at the edge's dst slot) is multiplied against the gathered source rows
G (dma_gather from a replicated node table), accumulating the weighted segment
sum directly in PSUM. No scatter-add, no all-reduce; two AllGathers replicate
x and h (bf16) between layers. Dense layers run as float32r (TF32-like) with
256-wide row pairs for full PE rate.

Single SPMD program; all per-core differences are pure data (indices, A
matrices, input slices). Block structure is made core-invariant by padding
each (quadrant, group) edge run to the max block count over cores.
"""
import os
import sys
import numpy as np

for _p in ("/opt/trn_rl_repo",):
    if _p not in sys.path:
        sys.path.insert(0, _p)

import ml_dtypes  # noqa: E402
from concourse import bacc, tile, mybir  # noqa: E402
from concourse.bass_utils import run_bass_kernel_spmd  # noqa: E402

BF16 = ml_dtypes.bfloat16

# problem constants (hardcoded per harness contract)
N_POL, N_TICK = 100000, 20000
N = N_POL + N_TICK
E = 1000000
POL_FEAT, EMB, HID, OUT_D = 7, 128, 256, 128
N_STATES = 60

NCORES = 8
OWN = 15000            # real rows per core
NG = 118               # row groups of 128 per core
NSH = NG * 128         # padded rows per core (15104)
NFULL = NCORES * NSH   # padded global rows (120832)
QUAD = NFULL // 4      # gather quadrant (30208 rows, < int16 max)
CH = 2048              # edges per gather chunk (16 blocks)
BLK_PER_CH = CH // 128
GT_ROWS = N_STATES + N_TICK + 4  # gather table (state_emb ++ emb_tick, padded)


def _wrap16(vals, width):
    """Pack 1-D int array into the [128, width] Q7 index layout: value j at
    [j%16, j//16], replicated across the 8 Q7-core stripes."""
    w = np.zeros((16, width), np.int16)
    j = np.arange(len(vals))
    w[j % 16, j // 16] = vals
    return np.tile(w, (8, 1))


def _host_plan(pol_features, state_ids, edge_index, edge_weight,
               W_proj, b_proj, state_emb, emb_tick,
               W1_rel, b1_rel, W1_root, W2_rel, b2_rel, W2_root):
    src = edge_index[0].astype(np.int64)
    dst = edge_index[1].astype(np.int64)
    w = edge_weight.astype(np.float32)

    core = dst // OWN                      # owner of the destination
    doff = dst - core * OWN
    g = doff // 128                        # local row-group
    slot = doff - g * 128                  # slot within group
    ps = (src // OWN) * NSH + (src % OWN)  # padded global source row
    q = ps // QUAD
    srel = (ps - q * QUAD).astype(np.int64)

    # counts per (core, quadrant, group)
    key = (core * 4 + q) * NG + g
    cnt = np.bincount(key, minlength=NCORES * 4 * NG).reshape(NCORES, 4, NG)
    B = -(-cnt // 128)                     # ceil
    B = B.max(axis=0)                      # [4, NG] uniform over cores

    # pad each quadrant's stream to a chunk multiple (extra blocks -> (q, NG-1))
    for qq in range(4):
        lq = int(B[qq].sum()) * 128
        pad = (-lq) % CH
        B[qq, NG - 1] += pad // 128
    S = B * 128                            # slots per (q, g)
    LTOT = int(S.sum())
    NB = LTOT // 128
    NCH = LTOT // CH

    # stream offsets per (q, g) in q-major order
    off = np.zeros((4, NG), np.int64)
    run = 0
    for qq in range(4):
        for gg in range(NG):
            off[qq, gg] = run
            run += S[qq, gg]

    # block map: for each block, (q, g, first-of-run, last-of-run)
    blocks = []
    for qq in range(4):
        for gg in range(NG):
            nb = int(B[qq, gg])
            for i in range(nb):
                blocks.append((qq, gg, i == 0, i == nb - 1))
    assert len(blocks) == NB

    plan = dict(LTOT=LTOT, NB=NB, NCH=NCH,
                blocks=blocks,
                chunk_q=[blocks[ci * BLK_PER_CH][0] for ci in range(NCH)])

    # ---- per-core edge arrays -------------------------------------------
    order = np.argsort(key, kind="stable")
    ks = key[order]
    starts = np.r_[0, np.cumsum(np.bincount(ks, minlength=NCORES * 4 * NG))]
    rank = np.arange(E) - starts[ks]
    off_flat = np.broadcast_to(off[None], (NCORES, 4, NG)).reshape(-1)
    jpos = off_flat[ks] + rank             # stream position within owner core

    eidx_list, A_list = [], []
    for c in range(NCORES):
        m = ks // (4 * NG) == c
        jj = jpos[m]
        sr = srel[order][m]
        sl = slot[order][m]
        ww = w[order][m]
        idx_stream = np.zeros(LTOT, np.int64)
        idx_stream[jj] = sr
        eidx_list.append(_wrap16(idx_stream, LTOT // 16))
        A = np.zeros((128, LTOT), np.float32)
        A[jj % 128, (jj // 128) * 128 + sl] = ww
        A_list.append(A.astype(BF16))

    # ---- per-core node-feature arrays -----------------------------------
    gtable = np.zeros((GT_ROWS, EMB), np.float32)
    gtable[:N_STATES] = state_emb
    gtable[N_STATES:N_STATES + N_TICK] = emb_tick

    polfT_list, sid_list, mask_list = [], [], []
    for c in range(NCORES):
        rows = c * OWN + np.arange(NSH)
        is_real = np.arange(NSH) < OWN
        is_pol = is_real & (rows < N_POL)
        is_tick = is_real & (rows >= N_POL) & (rows < N)
        pT = np.zeros((8, NSH), np.float32)
        pr = np.where(is_pol)[0]
        pT[:POL_FEAT, pr] = pol_features[rows[pr]].T
        pT[7, :] = 1.0
        polfT_list.append(pT)
        sid = np.zeros(NSH, np.int64)
        sid[pr] = state_ids[rows[pr]]
        tr = np.where(is_tick)[0]
        sid[tr] = N_STATES + (rows[tr] - N_POL)
        sid_list.append(_wrap16(sid, NSH // 16))
        mk = np.zeros((128, NG), np.float32)
        mk[np.arange(NSH) % 128, np.arange(NSH) // 128] = is_pol.astype(np.float32)
        mask_list.append(mk)

    shared = dict(
        gtable=gtable,
        Wp=np.concatenate([W_proj.astype(np.float32),
                           b_proj.astype(np.float32)[None, :]], axis=0),
        W1rel=W1_rel.astype(np.float32),
        W1root=W1_root.astype(np.float32),
        b1c=b1_rel.astype(np.float32).reshape(2, 128).T.copy(),
        W2rel=W2_rel.astype(np.float32).reshape(2, 128, 128),
        W2root=W2_root.astype(np.float32).reshape(2, 128, 128),
        b2c=b2_rel.astype(np.float32).reshape(128, 1),
        ident=np.eye(128, dtype=np.float32),
    )
    in_maps = []
    for c in range(NCORES):
        m = dict(shared)
        m.update(eidx=eidx_list[c], A=A_list[c], polfT=polfT_list[c],
                 sid=sid_list[c], mask=mask_list[c])
        in_maps.append(m)
    return plan, in_maps


def _build_nc(plan):
    PHASES = int(os.environ.get("K_PHASES", "99"))
    XMODE = os.environ.get("K_XMODE", "full")
    dt = mybir.dt
    f32, bf16, i16, f32r = dt.float32, dt.bfloat16, dt.int16, dt.float32r
    LTOT, NB, NCH = plan["LTOT"], plan["NB"], plan["NCH"]
    blocks, chunk_q = plan["blocks"], plan["chunk_q"]

    nc = bacc.Bacc("TRN2", target_bir_lowering=False, debug=False,
                   num_devices=NCORES)

    # inputs
    gtable = nc.dram_tensor("gtable", [GT_ROWS, EMB], f32, kind="ExternalInput")
    Wp = nc.dram_tensor("Wp", [8, 128], f32, kind="ExternalInput")
    W1rel = nc.dram_tensor("W1rel", [128, 256], f32, kind="ExternalInput")
    W1root = nc.dram_tensor("W1root", [128, 256], f32, kind="ExternalInput")
    b1c = nc.dram_tensor("b1c", [128, 2], f32, kind="ExternalInput")
    W2rel = nc.dram_tensor("W2rel", [2, 128, 128], f32, kind="ExternalInput")
    W2root = nc.dram_tensor("W2root", [2, 128, 128], f32, kind="ExternalInput")
    b2c = nc.dram_tensor("b2c", [128, 1], f32, kind="ExternalInput")
    ident = nc.dram_tensor("ident", [128, 128], f32, kind="ExternalInput")
    eidx = A = None
    if PHASES >= 3:
        eidx = nc.dram_tensor("eidx", [128, LTOT // 16], i16, kind="ExternalInput")
        A = nc.dram_tensor("A", [128, LTOT], bf16, kind="ExternalInput")
    polfT = nc.dram_tensor("polfT", [8, NSH], f32, kind="ExternalInput")
    sid = nc.dram_tensor("sid", [128, NSH // 16], i16, kind="ExternalInput")
    mask = nc.dram_tensor("mask", [128, NG], f32, kind="ExternalInput")

    out_own = nc.dram_tensor("out_own", [NSH, OUT_D], f32, kind="ExternalOutput")

    # internals
    x_own = nc.dram_tensor("x_own", [NSH, EMB], bf16)
    xT_own = nc.dram_tensor("xT_own", [128, NSH], bf16)
    x_full = nc.dram_tensor("x_full", [NFULL, EMB], bf16, addr_space="Shared")
    h_own = nc.dram_tensor("h_own", [NSH, HID], bf16)
    hT_own = nc.dram_tensor("hT_own", [2, 128, NSH], bf16)
    h_full = nc.dram_tensor("h_full", [NFULL, HID], bf16, addr_space="Shared")

    rg = [list(range(NCORES))]

    with tile.TileContext(nc) as tc:
        with (
            tc.tile_pool(name="const", bufs=1) as cp,
            tc.tile_pool(name="aggp", bufs=1) as aggp,
        ):
            # ---- constants -------------------------------------------------
            Wp_s = cp.tile([8, 128], f32)
            nc.sync.dma_start(Wp_s[:], Wp[:])
            W1rel_f = cp.tile([128, 256], f32)
            nc.sync.dma_start(W1rel_f[:], W1rel[:])
            W1rel_s = cp.tile([128, 256], bf16)
            nc.vector.tensor_copy(W1rel_s[:], W1rel_f[:])
            W1root_f = cp.tile([128, 256], f32)
            nc.sync.dma_start(W1root_f[:], W1root[:])
            W1root_s = cp.tile([128, 256], bf16)
            nc.vector.tensor_copy(W1root_s[:], W1root_f[:])
            b1_s = cp.tile([128, 2], f32)
            nc.sync.dma_start(b1_s[:], b1c[:])
            W2rel_f = cp.tile([128, 2, 128], f32)
            W2root_f = cp.tile([128, 2, 128], f32)
            for k in range(2):
                nc.sync.dma_start(W2rel_f[:, k, :], W2rel[k])
                nc.sync.dma_start(W2root_f[:, k, :], W2root[k])
            W2rel_s = cp.tile([128, 2, 128], bf16)
            W2root_s = cp.tile([128, 2, 128], bf16)
            nc.vector.tensor_copy(W2rel_s[:], W2rel_f[:])
            nc.vector.tensor_copy(W2root_s[:], W2root_f[:])
            b2_s = cp.tile([128, 1], f32)
            nc.sync.dma_start(b2_s[:], b2c[:])
            ident_s = cp.tile([128, 128], f32)
            nc.sync.dma_start(ident_s[:], ident[:])
            identb_s = cp.tile([128, 128], bf16)
            nc.vector.tensor_copy(identb_s[:], ident_s[:])
            mask_s = cp.tile([128, NG], f32)
            nc.sync.dma_start(mask_s[:], mask[:])

            # ---- build x_own (+ xT_own) -----------------------------------
            with (
                tc.tile_pool(name="xb_sb", bufs=2) as xsb,
                tc.tile_pool(name="xb_ps", bufs=2, space="PSUM") as xps,
            ):
                done = 0
                while done < NSH:
                    nt = min(BLK_PER_CH, (NSH - done) // 128)
                    nidx = nt * 128
                    sid_s = xsb.tile([128, CH // 16], i16, tag="sidc")
                    nc.sync.dma_start(sid_s[:, :nidx // 16],
                                      sid[:, done // 16:(done + nidx) // 16])
                    polfc = xsb.tile([8, CH], f32, tag="polfc")
                    nc.sync.dma_start(polfc[:, :nidx],
                                      polfT[:, done:done + nidx])
                    xg = xsb.tile([128, BLK_PER_CH, EMB], f32, tag="xg")
                    if XMODE != "mm":
                        nc.gpsimd.dma_gather(xg[:, :nt, :], gtable[:],
                                             sid_s[:, :nidx // 16], nidx, nidx, EMB,
                                             single_packet=False)
                    xrow = xsb.tile([128, BLK_PER_CH, EMB], bf16, tag="xrow")
                    for ti in range(nt):
                        t = done // 128 + ti
                        xf = xsb.tile([128, 128], f32, tag="xf")
                        if XMODE in ("full", "mm", "notrans"):
                            px = xps.tile([128, 128], f32, tag="px")
                            nc.tensor.matmul(px[:], polfc[:, ti * 128:(ti + 1) * 128],
                                             Wp_s[:], start=True, stop=True)
                            nc.scalar.activation(xf[:], px[:],
                                                 mybir.ActivationFunctionType.Relu)
                            nc.vector.tensor_scalar_mul(xf[:], xf[:], mask_s[:, t:t + 1])
                            if XMODE != "mm":
                                nc.vector.tensor_add(xf[:], xf[:], xg[:, ti, :])
                        else:  # gather-only
                            nc.vector.tensor_copy(xf[:], xg[:, ti, :])
                        nc.vector.tensor_copy(xrow[:, ti, :], xf[:])
                        if XMODE in ("full", "gather", "mm"):
                            pt = xps.tile([128, 128], f32, tag="ptx")
                            nc.tensor.transpose(pt[:], xf[:], ident_s[:])
                            xT_t = xsb.tile([128, 128], bf16, tag="xTt")
                            nc.vector.tensor_copy(xT_t[:], pt[:])
                            nc.sync.dma_start(xT_own[:, t * 128:(t + 1) * 128], xT_t[:])
                    xv = x_own[done:done + nidx, :].rearrange("(t p) e -> p t e", p=128)
                    nc.sync.dma_start(xv, xrow[:, :nt, :])
                    done += nidx

            if PHASES >= 2:
                if os.environ.get("K_NOCC"):
                    nc.sync.dma_start(x_full[:NSH, :], x_own[:])
                else:
                    nc.gpsimd.collective_compute(
                        "AllGather", mybir.AluOpType.bypass, replica_groups=rg,
                        ins=[x_own[:]], outs=[x_full[:]])

            # ---- edge phases ----------------------------------------------
            def edge_phase(layer, feat, src_full, agg, esb, eps):
                visited = set()
                cur = {}
                for ci in range(NCH):
                    qq = chunk_q[ci]
                    ei = esb.tile([128, 128], i16, tag="ei")
                    nc.sync.dma_start(ei[:], eidx[:, ci * 128:(ci + 1) * 128])
                    G = esb.tile([128, BLK_PER_CH, feat], bf16, tag="G")
                    nc.gpsimd.dma_gather(
                        G[:], src_full[qq * QUAD:(qq + 1) * QUAD, :],
                        ei[:], CH, CH, feat, single_packet=False)
                    Ac = esb.tile([128, CH], bf16, tag="Ac")
                    nc.sync.dma_start(Ac[:], A[:, ci * CH:(ci + 1) * CH])
                    for bi in range(BLK_PER_CH):
                        b = ci * BLK_PER_CH + bi
                        qb, gb, first, last = blocks[b]
                        if first:
                            cur[gb] = eps.tile([128, feat], f32, tag="ep",
                                               name=f"ep{layer}_{b}")
                        nc.tensor.matmul(cur[gb][:],
                                         Ac[:, bi * 128:(bi + 1) * 128],
                                         G[:, bi, :], start=first, stop=last)
                        if last:
                            dstv = agg[:, gb * feat:(gb + 1) * feat]
                            if gb in visited:
                                nc.vector.tensor_add(dstv, dstv, cur[gb][:])
                            else:
                                nc.vector.tensor_copy(dstv, cur[gb][:])
                                visited.add(gb)
                            del cur[gb]
                for gb in range(NG):
                    if gb not in visited:
                        nc.vector.memset(agg[:, gb * feat:(gb + 1) * feat], 0.0)

            if PHASES >= 3:
                agg1 = aggp.tile([128, NG * EMB], f32, tag="agg")
                with (
                    tc.tile_pool(name="e1_sb", bufs=2) as e1sb,
                    tc.tile_pool(name="e1_ps", bufs=4, space="PSUM") as e1ps,
                ):
                    edge_phase(1, EMB, x_full, agg1, e1sb, e1ps)

            if PHASES >= 4:
                # ---- dense layer 1 (row pairs, f32r) --------------------------
                with (
                    tc.tile_pool(name="d1_sb", bufs=2) as dsb,
                    tc.tile_pool(name="d1_pt", bufs=2, space="PSUM") as dpt,
                    tc.tile_pool(name="d1_ph", bufs=2, space="PSUM") as dph,
                ):
                    for pr in range(NG // 2):
                        gA = 2 * pr
                        aggT = dsb.tile([128, 256], bf16, tag="aggT")
                        for i in range(2):
                            pt = dpt.tile([128, 128], f32, tag="ptd")
                            nc.tensor.transpose(
                                pt[:], agg1[:, (gA + i) * 128:(gA + i + 1) * 128],
                                ident_s[:])
                            nc.vector.tensor_copy(aggT[:, i * 128:(i + 1) * 128], pt[:])
                        xTt = dsb.tile([128, 256], bf16, tag="xTt2")
                        nc.sync.dma_start(xTt[:], xT_own[:, gA * 128:(gA + 2) * 128])
                        hTt = dsb.tile([128, 2, 256], bf16, tag="hTt")
                        for hh in range(2):
                            ph = dph.tile([128, 256], f32, tag="ph")
                            nc.tensor.matmul(ph[:],
                                             W1rel_s[:, hh * 128:(hh + 1) * 128],
                                             aggT[:], start=True, stop=False)
                            nc.tensor.matmul(ph[:],
                                             W1root_s[:, hh * 128:(hh + 1) * 128],
                                             xTt[:], start=False, stop=True)
                            nc.scalar.activation(hTt[:, hh, :], ph[:],
                                                 mybir.ActivationFunctionType.Relu,
                                                 bias=b1_s[:, hh:hh + 1])
                            nc.sync.dma_start(hT_own[hh][:, gA * 128:(gA + 2) * 128],
                                              hTt[:, hh, :])
                        hrow = dsb.tile([128, 2, HID], bf16, tag="hrow")
                        for i in range(2):
                            for hh in range(2):
                                pt = dpt.tile([128, 128], bf16, tag="ptdb")
                                nc.tensor.transpose(pt[:],
                                                    hTt[:, hh, i * 128:(i + 1) * 128],
                                                    identb_s[:])
                                nc.vector.tensor_copy(
                                    hrow[:, i, hh * 128:(hh + 1) * 128], pt[:])
                        hv = h_own[gA * 128:(gA + 2) * 128, :].rearrange(
                            "(i p) d -> p i d", p=128)
                        nc.sync.dma_start(hv, hrow[:])

            if PHASES >= 5:
                if os.environ.get("K_NOCC"):
                    nc.sync.dma_start(h_full[:NSH, :], h_own[:])
                else:
                    nc.gpsimd.collective_compute(
                        "AllGather", mybir.AluOpType.bypass, replica_groups=rg,
                        ins=[h_own[:]], outs=[h_full[:]])

            if PHASES >= 6:
                agg2 = aggp.tile([128, NG * HID], bf16, tag="agg")
                with (
                    tc.tile_pool(name="e2_sb", bufs=2) as e2sb,
                    tc.tile_pool(name="e2_ps", bufs=4, space="PSUM") as e2ps,
                ):
                    edge_phase(2, HID, h_full, agg2, e2sb, e2ps)

            if PHASES < 7:
                with tc.tile_pool(name="dummy", bufs=1) as dup:
                    z = dup.tile([128, OUT_D], f32)
                    nc.vector.memset(z[:], 0.0)
                    for gg in range(NG):
                        ovd = out_own[gg * 128:(gg + 1) * 128, :]
                        nc.sync.dma_start(ovd, z[:])
            if PHASES >= 7:
                # ---- dense layer 2 (row pairs, f32r) --------------------------
                with (
                    tc.tile_pool(name="d2_sb", bufs=2) as dsb2,
                    tc.tile_pool(name="d2_pt", bufs=2, space="PSUM") as dpt2,
                    tc.tile_pool(name="d2_po", bufs=2, space="PSUM") as dpo2,
                ):
                    for pr in range(NG // 2):
                        gA = 2 * pr
                        aggT2 = dsb2.tile([128, 2, 256], bf16, tag="aggT2")
                        for i in range(2):
                            for k in range(2):
                                pt = dpt2.tile([128, 128], bf16, tag="ptb")
                                nc.tensor.transpose(
                                    pt[:],
                                    agg2[:, (gA + i) * HID + k * 128:
                                         (gA + i) * HID + (k + 1) * 128],
                                    identb_s[:])
                                nc.vector.tensor_copy(
                                    aggT2[:, k, i * 128:(i + 1) * 128], pt[:])
                        hTt2 = dsb2.tile([128, 2, 256], bf16, tag="hTt2")
                        for k in range(2):
                            nc.sync.dma_start(hTt2[:, k, :],
                                              hT_own[k][:, gA * 128:(gA + 2) * 128])
                        po = dpo2.tile([128, 256], f32, tag="po")
                        nc.tensor.matmul(po[:], W2rel_s[:, 0, :],
                                         aggT2[:, 0, :],
                                         start=True, stop=False)
                        nc.tensor.matmul(po[:], W2rel_s[:, 1, :],
                                         aggT2[:, 1, :],
                                         start=False, stop=False)
                        nc.tensor.matmul(po[:], W2root_s[:, 0, :],
                                         hTt2[:, 0, :],
                                         start=False, stop=False)
                        nc.tensor.matmul(po[:], W2root_s[:, 1, :],
                                         hTt2[:, 1, :],
                                         start=False, stop=True)
                        oT = dsb2.tile([128, 256], f32, tag="oT")
                        nc.vector.tensor_scalar_add(oT[:], po[:], b2_s[:, 0:1])
                        orow = dsb2.tile([128, 2, OUT_D], f32, tag="orow")
                        for i in range(2):
                            pt = dpt2.tile([128, 128], f32, tag="ptd2")
                            nc.tensor.transpose(pt[:], oT[:, i * 128:(i + 1) * 128],
                                                ident_s[:])
                            nc.vector.tensor_copy(orow[:, i, :], pt[:])
                        ov = out_own[gA * 128:(gA + 2) * 128, :].rearrange(
                            "(i p) e -> p i e", p=128)
                        nc.sync.dma_start(ov, orow[:])

    nc.compile()
    return nc


_CACHE = {}


def kernel(**inputs):
    inputs = {k: np.asarray(v) for k, v in inputs.items()}
    plan, in_maps = _host_plan(**inputs)
    key = (plan["LTOT"], tuple(plan["chunk_q"]), os.environ.get("K_PHASES", "99"), os.environ.get("K_XMODE", "full"), os.environ.get("K_NOCC", ""))
    if key not in _CACHE:
        _CACHE[key] = _build_nc(plan)
    nc = _CACHE[key]
    res = run_bass_kernel_spmd(nc, in_maps, list(range(NCORES)))
    out = np.empty((N, OUT_D), np.float32)
    for c in range(NCORES):
        out[c * OWN:(c + 1) * OWN] = res.results[c]["out_own"][:OWN]
    return out



# revision 10
# speedup vs baseline: 1886.4785x; 1886.4785x over previous
"""Trainium2 Bass kernel for BipartiteSAGE-style 2-layer GraphConv.

Reference computation (N=120000 nodes, E=1e6 edges, EMB=128, HID=256, OUT=128):
    pol = relu(pol_features @ W_proj + b_proj) + state_emb[state_ids]   [100000,128]
    x   = concat([pol, emb_tick])                                        [N,128]
    agg = segment_sum(x[src]*w, dst);  h = relu(agg@W1_rel + b1 + x@W1root)
    agg2= segment_sum(h[src]*w, dst);  out = agg2@W2_rel + b2 + h@W2root

Distribution: 8 NeuronCores, edges sharded by DESTINATION range (each core owns
15000 nodes). Per-edge aggregation runs as PE matmuls: for each 128-edge block,
a [128 edges, 128 dst-slot] selection matrix A (edge weight at the edge's dst
slot) is multiplied against gathered source rows G (dma_gather from a
replicated node table), accumulating the weighted segment sum in PSUM. Source
rows are indexed within 4 "quadrant" windows (two cores' rows each, int16-
indexable); x and h are replicated between layers by one AllGather each.

All large host-built operands of the original baseline are gone:
  - A matrices are built ON DEVICE per 128-edge block from compact per-edge
    (slot, weight) vectors via one DVE tensor_scalar: A = (iota == slot) * w.
  - The gather index stream ships as [16, L/16] (the Q7 16-partition wrap) and
    is replicated to the 8 stripes on device; kept SBUF-resident.
  - The x-gather table (state_emb ++ emb_tick) is assembled on device: tick
    embeddings ship sharded (2500 rows/core, bf16) and are AllGathered.
  - Weights/features ship bf16 where the math is bf16 anyway.
  - Edge gathers round-robin over 4 SWDGE queues.
  - Output ships bf16 and is upcast on host.

Single SPMD program; all per-core differences are pure data (indices, slots,
weights, input slices). Block structure is core-invariant: per (quadrant, dst
group) edge runs are padded to the max block count over cores.
"""
import os
import sys
import numpy as np

for _p in ("/opt/trn_rl_repo",):
    if _p not in sys.path:
        sys.path.insert(0, _p)

import ml_dtypes  # noqa: E402
from concourse import bacc, tile, mybir  # noqa: E402
from concourse.bass_utils import run_bass_kernel_spmd  # noqa: E402

BF16 = ml_dtypes.bfloat16

# problem constants (hardcoded per harness contract)
N_POL, N_TICK = 100000, 20000
N = N_POL + N_TICK
E = 1000000
POL_FEAT, EMB, HID, OUT_D = 7, 128, 256, 128
N_STATES = 60

NCORES = 8
OWN = 15000            # real rows per core
NG = 118               # row groups of 128 per core
NSH = NG * 128         # padded rows per core (15104)
NFULL = NCORES * NSH   # padded global rows (120832)
QUAD = NFULL // 4      # gather window: two cores' rows (30208 < int16 max)
CH = 2048              # edges per gather chunk (16 blocks)
BLK_PER_CH = CH // 128
NQUEUES = 4
GT_ROWS = 64 + N_TICK  # device-assembled gather table (state_emb ++ emb_tick)
TICK_SH = N_TICK // NCORES


def _wrap16(vals, width, dtype=np.int16):
    """[16, width] Q7 index layout: value j at [j%16, j//16]."""
    w = np.zeros((16, width), dtype)
    j = np.arange(len(vals))
    w[j % 16, j // 16] = vals
    return w


def _host_plan(pol_features, state_ids, edge_index, edge_weight,
               W_proj, b_proj, state_emb, emb_tick,
               W1_rel, b1_rel, W1_root, W2_rel, b2_rel, W2_root):
    src = edge_index[0].astype(np.int64)
    dst = edge_index[1].astype(np.int64)
    w = edge_weight.astype(np.float32)

    core = dst // OWN                      # owner of the destination
    doff = dst - core * OWN
    g = doff >> 7                          # local dst row-group
    slot = doff & 127                      # slot within group
    ps = (src // OWN) * NSH + (src % OWN)  # padded global source row
    q = ps // QUAD
    srel = (ps - q * QUAD).astype(np.int64)

    # counts per (core, quadrant, group)
    key = ((core * 4 + q) * NG + g).astype(np.int32)
    cnt = np.bincount(key, minlength=NCORES * 4 * NG).reshape(NCORES, 4, NG)
    B = -(-cnt // 128)                     # ceil
    B = B.max(axis=0)                      # [4, NG] uniform over cores

    # pad each quadrant's stream to a chunk multiple (extra blocks -> last group)
    for qq in range(4):
        lq = int(B[qq].sum()) * 128
        B[qq, NG - 1] += ((-lq) % CH) // 128
    S = B * 128                            # slots per (q, g)
    LTOT = int(S.sum())
    NB = LTOT // 128
    NCH = LTOT // CH

    # stream offsets per (q, g) in q-major order
    off = np.zeros((4, NG), np.int64)
    run = 0
    for qq in range(4):
        for gg in range(NG):
            off[qq, gg] = run
            run += S[qq, gg]

    blocks = []
    for qq in range(4):
        for gg in range(NG):
            nb = int(B[qq, gg])
            for i in range(nb):
                blocks.append((qq, gg, i == 0, i == nb - 1))
    assert len(blocks) == NB

    plan = dict(LTOT=LTOT, NB=NB, NCH=NCH, blocks=blocks,
                chunk_q=[blocks[ci * BLK_PER_CH][0] for ci in range(NCH)])

    # ---- per-core edge arrays (counting-sort into stream positions) ------
    order = np.argsort(key, kind="stable")
    ks = key[order]
    starts = np.r_[0, np.cumsum(np.bincount(ks, minlength=NCORES * 4 * NG))]
    rank = np.arange(E, dtype=np.int64) - starts[ks]
    off_flat = np.broadcast_to(off[None], (NCORES, 4, NG)).reshape(-1)
    jpos = off_flat[ks] + rank             # stream position within owner core
    srel_s = srel[order]
    slot_s = slot[order]
    w_s = w[order]
    core_bound = np.searchsorted(ks, np.arange(NCORES + 1) * (4 * NG))

    eidx_list, slot_list, wei_list = [], [], []
    for c in range(NCORES):
        lo, hi = core_bound[c], core_bound[c + 1]
        jj = jpos[lo:hi]
        ei = np.zeros((16, LTOT // 16), np.int16)
        ei[jj % 16, jj // 16] = srel_s[lo:hi]
        eidx_list.append(ei)
        sv = np.zeros((128, NB), BF16)
        sv[jj % 128, jj // 128] = slot_s[lo:hi]
        slot_list.append(sv)
        wv = np.zeros((128, NB), BF16)
        wv[jj % 128, jj // 128] = w_s[lo:hi]
        wei_list.append(wv)

    # ---- per-core node-feature arrays -----------------------------------
    polfT_list, sid_list, mask_list, tick_list = [], [], [], []
    for c in range(NCORES):
        rows = c * OWN + np.arange(NSH)
        is_real = np.arange(NSH) < OWN
        is_pol = is_real & (rows < N_POL)
        is_tick = is_real & (rows >= N_POL) & (rows < N)
        pT = np.zeros((8, NSH), BF16)
        pr = np.where(is_pol)[0]
        pT[:POL_FEAT, pr] = pol_features[rows[pr]].T.astype(BF16)
        pT[7, :] = 1.0
        polfT_list.append(pT)
        sid = np.zeros(NSH, np.int64)
        sid[pr] = state_ids[rows[pr]]
        tr = np.where(is_tick)[0]
        sid[tr] = 64 + (rows[tr] - N_POL)
        sid_list.append(_wrap16(sid, NSH // 16))
        mk = np.zeros((128, NG), np.float32)
        mk[np.arange(NSH) % 128, np.arange(NSH) // 128] = is_pol.astype(np.float32)
        mask_list.append(mk)
        tick_list.append(
            emb_tick[c * TICK_SH:(c + 1) * TICK_SH].astype(BF16))

    state_bf = np.zeros((64, EMB), BF16)
    state_bf[:N_STATES] = state_emb.astype(BF16)

    shared = dict(
        state_bf=state_bf,
        Wp=np.concatenate([W_proj.astype(BF16),
                           b_proj.astype(BF16)[None, :]], axis=0),
        W1rel=W1_rel.astype(BF16),
        W1root=W1_root.astype(BF16),
        b1c=b1_rel.astype(np.float32).reshape(2, 128).T.copy(),
        W2rel=W2_rel.astype(BF16).reshape(2, 128, 128),
        W2root=W2_root.astype(BF16).reshape(2, 128, 128),
        b2c=b2_rel.astype(np.float32).reshape(128, 1),
        ident=np.eye(128, dtype=np.float32),
    )
    in_maps = []
    for c in range(NCORES):
        m = dict(shared)
        m.update(eidx=eidx_list[c], slotv=slot_list[c], wei=wei_list[c],
                 polfT=polfT_list[c], sid=sid_list[c], mask=mask_list[c],
                 tick=tick_list[c])
        in_maps.append(m)
    return plan, in_maps


def _build_nc(plan):
    PHASES = int(os.environ.get("K_PHASES", "99"))
    dt = mybir.dt
    f32, bf16, i16, i32 = dt.float32, dt.bfloat16, dt.int16, dt.int32
    LTOT, NB, NCH = plan["LTOT"], plan["NB"], plan["NCH"]
    blocks, chunk_q = plan["blocks"], plan["chunk_q"]

    nc = bacc.Bacc("TRN2", target_bir_lowering=False, debug=False,
                   num_devices=NCORES, num_swdge_queues=NQUEUES)

    # inputs
    state_bf = nc.dram_tensor("state_bf", [64, EMB], bf16, kind="ExternalInput")
    tick = nc.dram_tensor("tick", [TICK_SH, EMB], bf16, kind="ExternalInput")
    Wp = nc.dram_tensor("Wp", [8, 128], bf16, kind="ExternalInput")
    W1rel = nc.dram_tensor("W1rel", [128, 256], bf16, kind="ExternalInput")
    W1root = nc.dram_tensor("W1root", [128, 256], bf16, kind="ExternalInput")
    b1c = nc.dram_tensor("b1c", [128, 2], f32, kind="ExternalInput")
    W2rel = nc.dram_tensor("W2rel", [2, 128, 128], bf16, kind="ExternalInput")
    W2root = nc.dram_tensor("W2root", [2, 128, 128], bf16, kind="ExternalInput")
    b2c = nc.dram_tensor("b2c", [128, 1], f32, kind="ExternalInput")
    ident = nc.dram_tensor("ident", [128, 128], f32, kind="ExternalInput")
    eidx = slotv = wei = None
    if PHASES >= 3:
        eidx = nc.dram_tensor("eidx", [16, LTOT // 16], i16, kind="ExternalInput")
        slotv = nc.dram_tensor("slotv", [128, NB], bf16, kind="ExternalInput")
        wei = nc.dram_tensor("wei", [128, NB], bf16, kind="ExternalInput")
    polfT = nc.dram_tensor("polfT", [8, NSH], bf16, kind="ExternalInput")
    sid = nc.dram_tensor("sid", [16, NSH // 16], i16, kind="ExternalInput")
    mask = nc.dram_tensor("mask", [128, NG], f32, kind="ExternalInput")

    out_own = nc.dram_tensor("out_own", [NSH, OUT_D], bf16, kind="ExternalOutput")

    # internals
    tick_i = nc.dram_tensor("tick_i", [TICK_SH, EMB], bf16)
    gt = nc.dram_tensor("gt", [GT_ROWS, EMB], bf16, addr_space="Shared")
    x_own = nc.dram_tensor("x_own", [NSH, EMB], bf16)
    xT_own = nc.dram_tensor("xT_own", [128, NSH], bf16)
    x_full = nc.dram_tensor("x_full", [NFULL, EMB], bf16, addr_space="Shared")
    h_own = nc.dram_tensor("h_own", [NSH, HID], bf16)
    hT_own = nc.dram_tensor("hT_own", [2, 128, NSH], bf16)
    h_full = nc.dram_tensor("h_full", [NFULL, HID], bf16, addr_space="Shared")

    rg = [list(range(NCORES))]

    def allgather(in_ap, out_ap):
        nc.gpsimd.collective_compute(
            "AllGather", mybir.AluOpType.bypass, replica_groups=rg,
            ins=[in_ap], outs=[out_ap])

    with tile.TileContext(nc) as tc:
        with (
            tc.tile_pool(name="const", bufs=1) as cp,
            tc.tile_pool(name="aggp", bufs=1) as aggp,
        ):
            # ---- constants -------------------------------------------------
            Wp_s = cp.tile([8, 128], bf16)
            nc.sync.dma_start(Wp_s[:], Wp[:])
            W1rel_s = cp.tile([128, 256], bf16)
            nc.sync.dma_start(W1rel_s[:], W1rel[:])
            W1root_s = cp.tile([128, 256], bf16)
            nc.sync.dma_start(W1root_s[:], W1root[:])
            b1_s = cp.tile([128, 2], f32)
            nc.sync.dma_start(b1_s[:], b1c[:])
            W2rel_s = cp.tile([128, 2, 128], bf16)
            W2root_s = cp.tile([128, 2, 128], bf16)
            for k in range(2):
                nc.sync.dma_start(W2rel_s[:, k, :], W2rel[k])
                nc.sync.dma_start(W2root_s[:, k, :], W2root[k])
            b2_s = cp.tile([128, 1], f32)
            nc.sync.dma_start(b2_s[:], b2c[:])
            ident_s = cp.tile([128, 128], f32)
            nc.sync.dma_start(ident_s[:], ident[:])
            identb_s = cp.tile([128, 128], bf16)
            nc.vector.tensor_copy(identb_s[:], ident_s[:])
            mask_s = cp.tile([128, NG], f32)
            nc.sync.dma_start(mask_s[:], mask[:])
            # free-dim iota 0..127, as f32 (for on-device A construction)
            io32 = cp.tile([128, 128], i32)
            nc.gpsimd.iota(io32[:], pattern=[[1, 128]], base=0,
                           channel_multiplier=0)
            ioF = cp.tile([128, 128], f32)
            nc.vector.tensor_copy(ioF[:], io32[:])

            # resident per-edge data: indices (replicated to 8 Q7 stripes),
            # dst slots and weights (f32 scalars for tensor_scalar)
            sidrep = cp.tile([128, NSH // 16], i16)
            for s in range(8):
                nc.sync.dma_start(sidrep[16 * s:16 * (s + 1), :], sid[:])
            if PHASES >= 3:
                eirep = cp.tile([128, LTOT // 16], i16)
                for s in range(8):
                    nc.sync.dma_start(eirep[16 * s:16 * (s + 1), :], eidx[:])
                slot_b = cp.tile([128, NB], bf16)
                nc.sync.dma_start(slot_b[:], slotv[:])
                slot_s = cp.tile([128, NB], f32)
                nc.vector.tensor_copy(slot_s[:], slot_b[:])
                wei_b = cp.tile([128, NB], bf16)
                nc.sync.dma_start(wei_b[:], wei[:])
                wei_s = cp.tile([128, NB], f32)
                nc.vector.tensor_copy(wei_s[:], wei_b[:])

            # ---- gather table: state_emb ++ AllGather(tick shards) ---------
            nc.sync.dma_start(gt[0:64, :], state_bf[:])
            nc.sync.dma_start(tick_i[:], tick[:])
            allgather(tick_i[:], gt[64:GT_ROWS, :])

            # ---- build x_own (+ xT_own) -----------------------------------
            with (
                tc.tile_pool(name="xb_sb", bufs=2) as xsb,
                tc.tile_pool(name="xb_ps", bufs=2, space="PSUM") as xps,
            ):
                done = 0
                while done < NSH:
                    nidx = min(CH, NSH - done)
                    nt = nidx // 128
                    xg = xsb.tile([128, BLK_PER_CH, EMB], bf16, tag="xg")
                    nc.gpsimd.dma_gather(
                        xg[:, :nt, :], gt[:],
                        sidrep[:, done // 16:(done + nidx) // 16],
                        nidx, nidx, EMB, single_packet=False)
                    polfc = xsb.tile([8, CH], bf16, tag="polfc")
                    nc.sync.dma_start(polfc[:, :nidx], polfT[:, done:done + nidx])
                    xrow = xsb.tile([128, BLK_PER_CH, EMB], bf16, tag="xrow")
                    for ti in range(nt):
                        t = done // 128 + ti
                        xf = xsb.tile([128, 128], f32, tag="xf")
                        px = xps.tile([128, 128], f32, tag="px")
                        nc.tensor.matmul(px[:], polfc[:, ti * 128:(ti + 1) * 128],
                                         Wp_s[:], start=True, stop=True)
                        nc.scalar.activation(xf[:], px[:],
                                             mybir.ActivationFunctionType.Relu)
                        nc.vector.tensor_scalar_mul(xf[:], xf[:], mask_s[:, t:t + 1])
                        nc.vector.tensor_add(xf[:], xf[:], xg[:, ti, :])
                        nc.vector.tensor_copy(xrow[:, ti, :], xf[:])
                        pt = xps.tile([128, 128], f32, tag="ptx")
                        nc.tensor.transpose(pt[:], xf[:], ident_s[:])
                        xT_t = xsb.tile([128, 128], bf16, tag="xTt")
                        nc.vector.tensor_copy(xT_t[:], pt[:])
                        nc.sync.dma_start(xT_own[:, t * 128:(t + 1) * 128], xT_t[:])
                    xv = x_own[done:done + nidx, :].rearrange("(t p) e -> p t e",
                                                              p=128)
                    nc.sync.dma_start(xv, xrow[:, :nt, :])
                    done += nidx

            if PHASES >= 2:
                allgather(x_own[:], x_full[:])

            # ---- edge phases ----------------------------------------------
            def edge_phase(layer, feat, src_full, agg, esb, eps):
                visited = set()
                cur = {}
                for ci in range(NCH):
                    qq = chunk_q[ci]
                    G = esb.tile([128, BLK_PER_CH, feat], bf16, tag="G")
                    nc.gpsimd.dma_gather(
                        G[:], src_full[qq * QUAD:(qq + 1) * QUAD, :],
                        eirep[:, ci * 128:(ci + 1) * 128], CH, CH, feat,
                        single_packet=False, queue_num=ci % NQUEUES)
                    for bi in range(BLK_PER_CH):
                        b = ci * BLK_PER_CH + bi
                        qb, gb, first, last = blocks[b]
                        Ab = esb.tile([128, 128], bf16, tag="Ab")
                        nc.vector.tensor_scalar(
                            Ab[:], ioF[:], slot_s[:, b:b + 1], wei_s[:, b:b + 1],
                            mybir.AluOpType.is_equal, mybir.AluOpType.mult)
                        if first:
                            cur[gb] = eps.tile([128, feat], f32, tag="ep",
                                               name=f"ep{layer}_{b}")
                        nc.tensor.matmul(cur[gb][:], Ab[:], G[:, bi, :],
                                         start=first, stop=last)
                        if last:
                            dstv = agg[:, gb * feat:(gb + 1) * feat]
                            if gb in visited:
                                nc.vector.tensor_add(dstv, dstv, cur[gb][:])
                            else:
                                nc.vector.tensor_copy(dstv, cur[gb][:])
                                visited.add(gb)
                            del cur[gb]
                for gb in range(NG):
                    if gb not in visited:
                        nc.vector.memset(agg[:, gb * feat:(gb + 1) * feat], 0.0)

            if PHASES >= 3:
                agg1 = aggp.tile([128, NG * EMB], f32, tag="agg")
                with (
                    tc.tile_pool(name="e1_sb", bufs=3) as e1sb,
                    tc.tile_pool(name="e1_ps", bufs=4, space="PSUM") as e1ps,
                ):
                    edge_phase(1, EMB, x_full, agg1, e1sb, e1ps)

            if PHASES >= 4:
                # ---- dense layer 1 (row pairs) ------------------------------
                with (
                    tc.tile_pool(name="d1_sb", bufs=2) as dsb,
                    tc.tile_pool(name="d1_pt", bufs=2, space="PSUM") as dpt,
                    tc.tile_pool(name="d1_ph", bufs=2, space="PSUM") as dph,
                ):
                    for pr in range(NG // 2):
                        gA = 2 * pr
                        aggT = dsb.tile([128, 256], bf16, tag="aggT")
                        for i in range(2):
                            pt = dpt.tile([128, 128], f32, tag="ptd")
                            nc.tensor.transpose(
                                pt[:], agg1[:, (gA + i) * 128:(gA + i + 1) * 128],
                                ident_s[:])
                            nc.vector.tensor_copy(aggT[:, i * 128:(i + 1) * 128], pt[:])
                        xTt = dsb.tile([128, 256], bf16, tag="xTt2")
                        nc.sync.dma_start(xTt[:], xT_own[:, gA * 128:(gA + 2) * 128])
                        hTt = dsb.tile([128, 2, 256], bf16, tag="hTt")
                        for hh in range(2):
                            ph = dph.tile([128, 256], f32, tag="ph")
                            nc.tensor.matmul(ph[:],
                                             W1rel_s[:, hh * 128:(hh + 1) * 128],
                                             aggT[:], start=True, stop=False)
                            nc.tensor.matmul(ph[:],
                                             W1root_s[:, hh * 128:(hh + 1) * 128],
                                             xTt[:], start=False, stop=True)
                            nc.scalar.activation(hTt[:, hh, :], ph[:],
                                                 mybir.ActivationFunctionType.Relu,
                                                 bias=b1_s[:, hh:hh + 1])
                            nc.sync.dma_start(hT_own[hh][:, gA * 128:(gA + 2) * 128],
                                              hTt[:, hh, :])
                        hrow = dsb.tile([128, 2, HID], bf16, tag="hrow")
                        for i in range(2):
                            for hh in range(2):
                                pt = dpt.tile([128, 128], bf16, tag="ptdb")
                                nc.tensor.transpose(pt[:],
                                                    hTt[:, hh, i * 128:(i + 1) * 128],
                                                    identb_s[:])
                                nc.vector.tensor_copy(
                                    hrow[:, i, hh * 128:(hh + 1) * 128], pt[:])
                        hv = h_own[gA * 128:(gA + 2) * 128, :].rearrange(
                            "(i p) d -> p i d", p=128)
                        nc.sync.dma_start(hv, hrow[:])

            if PHASES >= 5:
                allgather(h_own[:], h_full[:])

            if PHASES >= 6:
                agg2 = aggp.tile([128, NG * HID], bf16, tag="agg")
                with (
                    tc.tile_pool(name="e2_sb", bufs=3) as e2sb,
                    tc.tile_pool(name="e2_ps", bufs=4, space="PSUM") as e2ps,
                ):
                    edge_phase(2, HID, h_full, agg2, e2sb, e2ps)

            if PHASES < 7:
                with tc.tile_pool(name="dummy", bufs=1) as dup:
                    z = dup.tile([128, OUT_D], bf16)
                    nc.vector.memset(z[:], 0.0)
                    for gg in range(NG):
                        ovd = out_own[gg * 128:(gg + 1) * 128, :]
                        nc.sync.dma_start(ovd, z[:])
            if PHASES >= 7:
                # ---- dense layer 2 (row pairs) ------------------------------
                with (
                    tc.tile_pool(name="d2_sb", bufs=2) as dsb2,
                    tc.tile_pool(name="d2_pt", bufs=2, space="PSUM") as dpt2,
                    tc.tile_pool(name="d2_po", bufs=2, space="PSUM") as dpo2,
                ):
                    for pr in range(NG // 2):
                        gA = 2 * pr
                        aggT2 = dsb2.tile([128, 2, 256], bf16, tag="aggT2")
                        for i in range(2):
                            for k in range(2):
                                pt = dpt2.tile([128, 128], bf16, tag="ptb")
                                nc.tensor.transpose(
                                    pt[:],
                                    agg2[:, (gA + i) * HID + k * 128:
                                         (gA + i) * HID + (k + 1) * 128],
                                    identb_s[:])
                                nc.vector.tensor_copy(
                                    aggT2[:, k, i * 128:(i + 1) * 128], pt[:])
                        hTt2 = dsb2.tile([128, 2, 256], bf16, tag="hTt2")
                        for k in range(2):
                            nc.sync.dma_start(hTt2[:, k, :],
                                              hT_own[k][:, gA * 128:(gA + 2) * 128])
                        po = dpo2.tile([128, 256], f32, tag="po")
                        nc.tensor.matmul(po[:], W2rel_s[:, 0, :],
                                         aggT2[:, 0, :],
                                         start=True, stop=False)
                        nc.tensor.matmul(po[:], W2rel_s[:, 1, :],
                                         aggT2[:, 1, :],
                                         start=False, stop=False)
                        nc.tensor.matmul(po[:], W2root_s[:, 0, :],
                                         hTt2[:, 0, :],
                                         start=False, stop=False)
                        nc.tensor.matmul(po[:], W2root_s[:, 1, :],
                                         hTt2[:, 1, :],
                                         start=False, stop=True)
                        oT = dsb2.tile([128, 256], f32, tag="oT")
                        nc.vector.tensor_scalar_add(oT[:], po[:], b2_s[:, 0:1])
                        orow = dsb2.tile([128, 2, OUT_D], bf16, tag="orow")
                        for i in range(2):
                            pt = dpt2.tile([128, 128], f32, tag="ptd2")
                            nc.tensor.transpose(pt[:], oT[:, i * 128:(i + 1) * 128],
                                                ident_s[:])
                            nc.vector.tensor_copy(orow[:, i, :], pt[:])
                        ov = out_own[gA * 128:(gA + 2) * 128, :].rearrange(
                            "(i p) e -> p i e", p=128)
                        nc.sync.dma_start(ov, orow[:])

    nc.compile()
    return nc


_CACHE = {}


def kernel(**inputs):
    inputs = {k: np.asarray(v) for k, v in inputs.items()}
    plan, in_maps = _host_plan(**inputs)
    key = (plan["LTOT"], tuple(plan["chunk_q"]),
           os.environ.get("K_PHASES", "99"))
    if key not in _CACHE:
        _CACHE[key] = _build_nc(plan)
    nc = _CACHE[key]
    res = run_bass_kernel_spmd(nc, in_maps, list(range(NCORES)))
    out = np.empty((N, OUT_D), np.float32)
    for c in range(NCORES):
        out[c * OWN:(c + 1) * OWN] = res.results[c]["out_own"][:OWN].astype(np.float32)
    return out


# revision 12
# speedup vs baseline: 2136.6807x; 1.1326x over previous
"""Trainium2 Bass kernel for BipartiteSAGE-style 2-layer GraphConv.

Reference computation (N=120000 nodes, E=1e6 edges, EMB=128, HID=256, OUT=128):
    pol = relu(pol_features @ W_proj + b_proj) + state_emb[state_ids]   [100000,128]
    x   = concat([pol, emb_tick])                                        [N,128]
    agg = segment_sum(x[src]*w, dst);  h = relu(agg@W1_rel + b1 + x@W1root)
    agg2= segment_sum(h[src]*w, dst);  out = agg2@W2_rel + b2 + h@W2root

Distribution: 8 NeuronCores, edges sharded by DESTINATION range (each core owns
15000 nodes). Per-edge aggregation runs as PE matmuls: for each 128-edge block,
a [128 edges, 128 dst-slot] selection matrix A (edge weight at the edge's dst
slot) is multiplied against gathered source rows G (dma_gather from a
replicated node table), accumulating the weighted segment sum in PSUM. Source
rows are indexed within 4 "quadrant" windows (two cores' rows each, int16-
indexable); x and h are replicated between layers by one AllGather each.

All large host-built operands of the original baseline are gone:
  - A matrices are built ON DEVICE per 128-edge block from compact per-edge
    (slot, weight) vectors via one DVE tensor_scalar: A = (iota == slot) * w.
  - The gather index stream ships as [16, L/16] (the Q7 16-partition wrap) and
    is replicated to the 8 stripes on device; kept SBUF-resident.
  - The x-gather table (state_emb ++ emb_tick) is assembled on device: tick
    embeddings ship sharded (2500 rows/core, bf16) and are AllGathered.
  - Weights/features ship bf16 where the math is bf16 anyway.
  - Edge gathers round-robin over 4 SWDGE queues.
  - Output ships bf16 and is upcast on host.

Single SPMD program; all per-core differences are pure data (indices, slots,
weights, input slices). Block structure is core-invariant: per (quadrant, dst
group) edge runs are padded to the max block count over cores.
"""
import os
import sys
import numpy as np

for _p in ("/opt/trn_rl_repo",):
    if _p not in sys.path:
        sys.path.insert(0, _p)

import ml_dtypes  # noqa: E402
from concourse import bacc, tile, mybir  # noqa: E402
from concourse.bass_utils import run_bass_kernel_spmd  # noqa: E402

BF16 = ml_dtypes.bfloat16

# problem constants (hardcoded per harness contract)
N_POL, N_TICK = 100000, 20000
N = N_POL + N_TICK
E = 1000000
POL_FEAT, EMB, HID, OUT_D = 7, 128, 256, 128
N_STATES = 60

NCORES = 8
OWN = 15000            # real rows per core
NG = 118               # row groups of 128 per core
NSH = NG * 128         # padded rows per core (15104)
NFULL = NCORES * NSH   # padded global rows (120832)
QUAD = NFULL // 4      # gather window: two cores' rows (30208 < int16 max)
CH = int(os.environ.get("K_CH", "2048"))  # edges per gather chunk
BLK_PER_CH = CH // 128
NQUEUES = 4
GT_ROWS = 64 + N_TICK  # device-assembled gather table (state_emb ++ emb_tick)
TICK_SH = N_TICK // NCORES


def _wrap16(vals, width, dtype=np.int16):
    """[16, width] Q7 index layout: value j at [j%16, j//16]."""
    w = np.zeros((16, width), dtype)
    j = np.arange(len(vals))
    w[j % 16, j // 16] = vals
    return w


def _host_plan(pol_features, state_ids, edge_index, edge_weight,
               W_proj, b_proj, state_emb, emb_tick,
               W1_rel, b1_rel, W1_root, W2_rel, b2_rel, W2_root):
    src = edge_index[0].astype(np.int64)
    dst = edge_index[1].astype(np.int64)
    w = edge_weight.astype(np.float32)

    core = dst // OWN                      # owner of the destination
    doff = dst - core * OWN
    g = doff >> 7                          # local dst row-group
    slot = doff & 127                      # slot within group
    ps = (src // OWN) * NSH + (src % OWN)  # padded global source row
    q = ps // QUAD
    srel = (ps - q * QUAD).astype(np.int64)

    # counts per (core, quadrant, group)
    key = ((core * 4 + q) * NG + g).astype(np.int32)
    cnt = np.bincount(key, minlength=NCORES * 4 * NG).reshape(NCORES, 4, NG)
    B = -(-cnt // 128)                     # ceil
    B = B.max(axis=0)                      # [4, NG] uniform over cores

    # pad each quadrant's stream to a chunk multiple (extra blocks -> last group)
    for qq in range(4):
        lq = int(B[qq].sum()) * 128
        B[qq, NG - 1] += ((-lq) % CH) // 128
    S = B * 128                            # slots per (q, g)
    LTOT = int(S.sum())
    NB = LTOT // 128
    NCH = LTOT // CH

    # stream offsets per (q, g) in q-major order
    off = np.zeros((4, NG), np.int64)
    run = 0
    for qq in range(4):
        for gg in range(NG):
            off[qq, gg] = run
            run += S[qq, gg]

    blocks = []
    for qq in range(4):
        for gg in range(NG):
            nb = int(B[qq, gg])
            for i in range(nb):
                blocks.append((qq, gg, i == 0, i == nb - 1))
    assert len(blocks) == NB

    plan = dict(LTOT=LTOT, NB=NB, NCH=NCH, blocks=blocks,
                chunk_q=[blocks[ci * BLK_PER_CH][0] for ci in range(NCH)])

    # ---- per-core edge arrays (counting-sort into stream positions) ------
    order = np.argsort(key, kind="stable")
    ks = key[order]
    starts = np.r_[0, np.cumsum(np.bincount(ks, minlength=NCORES * 4 * NG))]
    rank = np.arange(E, dtype=np.int64) - starts[ks]
    off_flat = np.broadcast_to(off[None], (NCORES, 4, NG)).reshape(-1)
    jpos = off_flat[ks] + rank             # stream position within owner core
    srel_s = srel[order]
    slot_s = slot[order]
    w_s = w[order]
    core_bound = np.searchsorted(ks, np.arange(NCORES + 1) * (4 * NG))

    eidx_list, slot_list, wei_list = [], [], []
    for c in range(NCORES):
        lo, hi = core_bound[c], core_bound[c + 1]
        jj = jpos[lo:hi]
        ei = np.zeros((16, LTOT // 16), np.int16)
        ei[jj % 16, jj // 16] = srel_s[lo:hi]
        eidx_list.append(ei)
        sv = np.zeros((128, NB), BF16)
        sv[jj % 128, jj // 128] = slot_s[lo:hi]
        slot_list.append(sv)
        wv = np.zeros((128, NB), BF16)
        wv[jj % 128, jj // 128] = w_s[lo:hi]
        wei_list.append(wv)

    # ---- per-core node-feature arrays -----------------------------------
    polfT_list, sid_list, mask_list, tick_list = [], [], [], []
    for c in range(NCORES):
        rows = c * OWN + np.arange(NSH)
        is_real = np.arange(NSH) < OWN
        is_pol = is_real & (rows < N_POL)
        is_tick = is_real & (rows >= N_POL) & (rows < N)
        pT = np.zeros((8, NSH), BF16)
        pr = np.where(is_pol)[0]
        pT[:POL_FEAT, pr] = pol_features[rows[pr]].T.astype(BF16)
        pT[7, :] = 1.0
        polfT_list.append(pT)
        sid = np.zeros(NSH, np.int64)
        sid[pr] = state_ids[rows[pr]]
        tr = np.where(is_tick)[0]
        sid[tr] = 64 + (rows[tr] - N_POL)
        sid_list.append(_wrap16(sid, NSH // 16))
        mk = np.zeros((128, NG), np.float32)
        mk[np.arange(NSH) % 128, np.arange(NSH) // 128] = is_pol.astype(np.float32)
        mask_list.append(mk)
        tick_list.append(
            emb_tick[c * TICK_SH:(c + 1) * TICK_SH].astype(BF16))

    state_bf = np.zeros((64, EMB), BF16)
    state_bf[:N_STATES] = state_emb.astype(BF16)

    shared = dict(
        state_bf=state_bf,
        Wp=np.concatenate([W_proj.astype(BF16),
                           b_proj.astype(BF16)[None, :]], axis=0),
        W1rel=W1_rel.astype(BF16),
        W1root=W1_root.astype(BF16),
        b1c=b1_rel.astype(np.float32).reshape(2, 128).T.copy(),
        W2rel=W2_rel.astype(BF16).reshape(2, 128, 128),
        W2root=W2_root.astype(BF16).reshape(2, 128, 128),
        b2c=b2_rel.astype(np.float32).reshape(128, 1),
        ident=np.eye(128, dtype=np.float32),
    )
    in_maps = []
    for c in range(NCORES):
        m = dict(shared)
        m.update(eidx=eidx_list[c], slotv=slot_list[c], wei=wei_list[c],
                 polfT=polfT_list[c], sid=sid_list[c], mask=mask_list[c],
                 tick=tick_list[c])
        in_maps.append(m)
    return plan, in_maps


def _build_nc(plan):
    PHASES = int(os.environ.get("K_PHASES", "99"))
    dt = mybir.dt
    f32, bf16, i16, i32 = dt.float32, dt.bfloat16, dt.int16, dt.int32
    LTOT, NB, NCH = plan["LTOT"], plan["NB"], plan["NCH"]
    blocks, chunk_q = plan["blocks"], plan["chunk_q"]

    nc = bacc.Bacc("TRN2", target_bir_lowering=False, debug=False,
                   num_devices=NCORES, num_swdge_queues=NQUEUES)

    # inputs
    state_bf = nc.dram_tensor("state_bf", [64, EMB], bf16, kind="ExternalInput")
    tick = nc.dram_tensor("tick", [TICK_SH, EMB], bf16, kind="ExternalInput")
    Wp = nc.dram_tensor("Wp", [8, 128], bf16, kind="ExternalInput")
    W1rel = nc.dram_tensor("W1rel", [128, 256], bf16, kind="ExternalInput")
    W1root = nc.dram_tensor("W1root", [128, 256], bf16, kind="ExternalInput")
    b1c = nc.dram_tensor("b1c", [128, 2], f32, kind="ExternalInput")
    W2rel = nc.dram_tensor("W2rel", [2, 128, 128], bf16, kind="ExternalInput")
    W2root = nc.dram_tensor("W2root", [2, 128, 128], bf16, kind="ExternalInput")
    b2c = nc.dram_tensor("b2c", [128, 1], f32, kind="ExternalInput")
    ident = nc.dram_tensor("ident", [128, 128], f32, kind="ExternalInput")
    eidx = slotv = wei = None
    if PHASES >= 3:
        eidx = nc.dram_tensor("eidx", [16, LTOT // 16], i16, kind="ExternalInput")
        slotv = nc.dram_tensor("slotv", [128, NB], bf16, kind="ExternalInput")
        wei = nc.dram_tensor("wei", [128, NB], bf16, kind="ExternalInput")
    polfT = nc.dram_tensor("polfT", [8, NSH], bf16, kind="ExternalInput")
    sid = nc.dram_tensor("sid", [16, NSH // 16], i16, kind="ExternalInput")
    mask = nc.dram_tensor("mask", [128, NG], f32, kind="ExternalInput")

    out_own = nc.dram_tensor("out_own", [NSH, OUT_D], bf16, kind="ExternalOutput")

    # internals
    tick_i = nc.dram_tensor("tick_i", [TICK_SH, EMB], bf16)
    gt = nc.dram_tensor("gt", [GT_ROWS, EMB], bf16, addr_space="Shared")
    x_own = nc.dram_tensor("x_own", [NSH, EMB], bf16)
    xT_own = nc.dram_tensor("xT_own", [128, NSH], bf16)
    x_full = nc.dram_tensor("x_full", [NFULL, EMB], bf16, addr_space="Shared")
    h_own = nc.dram_tensor("h_own", [NSH, HID], bf16)
    hT_own = nc.dram_tensor("hT_own", [2, 128, NSH], bf16)
    h_full = nc.dram_tensor("h_full", [NFULL, HID], bf16, addr_space="Shared")

    rg = [list(range(NCORES))]

    def allgather(in_ap, out_ap):
        nc.gpsimd.collective_compute(
            "AllGather", mybir.AluOpType.bypass, replica_groups=rg,
            ins=[in_ap], outs=[out_ap])

    with tile.TileContext(nc) as tc:
        with (
            tc.tile_pool(name="const", bufs=1) as cp,
            tc.tile_pool(name="aggp", bufs=1) as aggp,
        ):
            # ---- constants -------------------------------------------------
            Wp_s = cp.tile([8, 128], bf16)
            nc.sync.dma_start(Wp_s[:], Wp[:])
            W1rel_s = cp.tile([128, 256], bf16)
            nc.sync.dma_start(W1rel_s[:], W1rel[:])
            W1root_s = cp.tile([128, 256], bf16)
            nc.sync.dma_start(W1root_s[:], W1root[:])
            b1_s = cp.tile([128, 2], f32)
            nc.sync.dma_start(b1_s[:], b1c[:])
            W2rel_s = cp.tile([128, 2, 128], bf16)
            W2root_s = cp.tile([128, 2, 128], bf16)
            for k in range(2):
                nc.sync.dma_start(W2rel_s[:, k, :], W2rel[k])
                nc.sync.dma_start(W2root_s[:, k, :], W2root[k])
            b2_s = cp.tile([128, 1], f32)
            nc.sync.dma_start(b2_s[:], b2c[:])
            ident_s = cp.tile([128, 128], f32)
            nc.sync.dma_start(ident_s[:], ident[:])
            identb_s = cp.tile([128, 128], bf16)
            nc.vector.tensor_copy(identb_s[:], ident_s[:])
            mask_s = cp.tile([128, NG], f32)
            nc.sync.dma_start(mask_s[:], mask[:])
            # free-dim iota 0..127, as f32 (for on-device A construction)
            io32 = cp.tile([128, 128], i32)
            nc.gpsimd.iota(io32[:], pattern=[[1, 128]], base=0,
                           channel_multiplier=0)
            ioF = cp.tile([128, 128], f32)
            nc.vector.tensor_copy(ioF[:], io32[:])

            # resident per-edge data: indices (replicated to 8 Q7 stripes),
            # dst slots and weights (f32 scalars for tensor_scalar)
            sidrep = cp.tile([128, NSH // 16], i16)
            for s in range(8):
                nc.sync.dma_start(sidrep[16 * s:16 * (s + 1), :], sid[:])
            if PHASES >= 3:
                eirep = cp.tile([128, LTOT // 16], i16)
                for s in range(8):
                    nc.sync.dma_start(eirep[16 * s:16 * (s + 1), :], eidx[:])
                slot_b = cp.tile([128, NB], bf16)
                nc.sync.dma_start(slot_b[:], slotv[:])
                slot_s = cp.tile([128, NB], f32)
                nc.vector.tensor_copy(slot_s[:], slot_b[:])
                wei_b = cp.tile([128, NB], bf16)
                nc.sync.dma_start(wei_b[:], wei[:])
                wei_s = cp.tile([128, NB], f32)
                nc.vector.tensor_copy(wei_s[:], wei_b[:])

            # ---- gather table: state_emb ++ AllGather(tick shards) ---------
            nc.sync.dma_start(gt[0:64, :], state_bf[:])
            nc.sync.dma_start(tick_i[:], tick[:])
            allgather(tick_i[:], gt[64:GT_ROWS, :])

            # ---- build x_own (+ xT_own) -----------------------------------
            with (
                tc.tile_pool(name="xb_sb", bufs=2) as xsb,
                tc.tile_pool(name="xb_ps", bufs=2, space="PSUM") as xps,
            ):
                done = 0
                while done < NSH:
                    nidx = min(CH, NSH - done)
                    nt = nidx // 128
                    xg = xsb.tile([128, BLK_PER_CH, EMB], bf16, tag="xg")
                    nc.gpsimd.dma_gather(
                        xg[:, :nt, :], gt[:],
                        sidrep[:, done // 16:(done + nidx) // 16],
                        nidx, nidx, EMB, single_packet=False)
                    polfc = xsb.tile([8, CH], bf16, tag="polfc")
                    nc.sync.dma_start(polfc[:, :nidx], polfT[:, done:done + nidx])
                    xrow = xsb.tile([128, BLK_PER_CH, EMB], bf16, tag="xrow")
                    for ti in range(nt):
                        t = done // 128 + ti
                        xf = xsb.tile([128, 128], f32, tag="xf")
                        px = xps.tile([128, 128], f32, tag="px")
                        nc.tensor.matmul(px[:], polfc[:, ti * 128:(ti + 1) * 128],
                                         Wp_s[:], start=True, stop=True)
                        nc.scalar.activation(xf[:], px[:],
                                             mybir.ActivationFunctionType.Relu)
                        nc.vector.tensor_scalar_mul(xf[:], xf[:], mask_s[:, t:t + 1])
                        nc.vector.tensor_add(xf[:], xf[:], xg[:, ti, :])
                        nc.vector.tensor_copy(xrow[:, ti, :], xf[:])
                        pt = xps.tile([128, 128], f32, tag="ptx")
                        nc.tensor.transpose(pt[:], xf[:], ident_s[:])
                        xT_t = xsb.tile([128, 128], bf16, tag="xTt")
                        nc.vector.tensor_copy(xT_t[:], pt[:])
                        nc.sync.dma_start(xT_own[:, t * 128:(t + 1) * 128], xT_t[:])
                    xv = x_own[done:done + nidx, :].rearrange("(t p) e -> p t e",
                                                              p=128)
                    nc.sync.dma_start(xv, xrow[:, :nt, :])
                    done += nidx

            if PHASES >= 2:
                allgather(x_own[:], x_full[:])

            # ---- edge phases ----------------------------------------------
            def edge_phase(layer, feat, src_full, agg, esb, eps):
                visited = set()
                cur = {}
                for ci in range(NCH):
                    qq = chunk_q[ci]
                    G = esb.tile([128, BLK_PER_CH, feat], bf16, tag="G")
                    nc.gpsimd.dma_gather(
                        G[:], src_full[qq * QUAD:(qq + 1) * QUAD, :],
                        eirep[:, ci * (CH // 16):(ci + 1) * (CH // 16)],
                        CH, CH, feat,
                        single_packet=False, queue_num=ci % NQUEUES)
                    for bi in range(BLK_PER_CH):
                        b = ci * BLK_PER_CH + bi
                        qb, gb, first, last = blocks[b]
                        Ab = esb.tile([128, 128], bf16, tag="Ab")
                        nc.vector.tensor_scalar(
                            Ab[:], ioF[:], slot_s[:, b:b + 1], wei_s[:, b:b + 1],
                            mybir.AluOpType.is_equal, mybir.AluOpType.mult)
                        if first:
                            cur[gb] = eps.tile([128, feat], f32, tag="ep",
                                               name=f"ep{layer}_{b}")
                        nc.tensor.matmul(cur[gb][:], Ab[:], G[:, bi, :],
                                         start=first, stop=last)
                        if last:
                            dstv = agg[:, gb * feat:(gb + 1) * feat]
                            if gb in visited:
                                nc.vector.tensor_add(dstv, dstv, cur[gb][:])
                            else:
                                nc.vector.tensor_copy(dstv, cur[gb][:])
                                visited.add(gb)
                            del cur[gb]
                for gb in range(NG):
                    if gb not in visited:
                        nc.vector.memset(agg[:, gb * feat:(gb + 1) * feat], 0.0)

            if PHASES >= 3:
                agg1 = aggp.tile([128, NG * EMB], f32, tag="agg")
                with (
                    tc.tile_pool(name="e1_sb", bufs=4) as e1sb,
                    tc.tile_pool(name="e1_ps", bufs=4, space="PSUM") as e1ps,
                ):
                    edge_phase(1, EMB, x_full, agg1, e1sb, e1ps)

            if PHASES >= 4:
                # ---- dense layer 1 (row pairs) ------------------------------
                with (
                    tc.tile_pool(name="d1_sb", bufs=2) as dsb,
                    tc.tile_pool(name="d1_pt", bufs=2, space="PSUM") as dpt,
                    tc.tile_pool(name="d1_ph", bufs=2, space="PSUM") as dph,
                ):
                    for pr in range(NG // 2):
                        gA = 2 * pr
                        aggT = dsb.tile([128, 256], bf16, tag="aggT")
                        for i in range(2):
                            pt = dpt.tile([128, 128], f32, tag="ptd")
                            nc.tensor.transpose(
                                pt[:], agg1[:, (gA + i) * 128:(gA + i + 1) * 128],
                                ident_s[:])
                            nc.vector.tensor_copy(aggT[:, i * 128:(i + 1) * 128], pt[:])
                        xTt = dsb.tile([128, 256], bf16, tag="xTt2")
                        nc.sync.dma_start(xTt[:], xT_own[:, gA * 128:(gA + 2) * 128])
                        hTt = dsb.tile([128, 2, 256], bf16, tag="hTt")
                        for hh in range(2):
                            ph = dph.tile([128, 256], f32, tag="ph")
                            nc.tensor.matmul(ph[:],
                                             W1rel_s[:, hh * 128:(hh + 1) * 128],
                                             aggT[:], start=True, stop=False)
                            nc.tensor.matmul(ph[:],
                                             W1root_s[:, hh * 128:(hh + 1) * 128],
                                             xTt[:], start=False, stop=True)
                            nc.scalar.activation(hTt[:, hh, :], ph[:],
                                                 mybir.ActivationFunctionType.Relu,
                                                 bias=b1_s[:, hh:hh + 1])
                            nc.sync.dma_start(hT_own[hh][:, gA * 128:(gA + 2) * 128],
                                              hTt[:, hh, :])
                        hrow = dsb.tile([128, 2, HID], bf16, tag="hrow")
                        for i in range(2):
                            for hh in range(2):
                                pt = dpt.tile([128, 128], bf16, tag="ptdb")
                                nc.tensor.transpose(pt[:],
                                                    hTt[:, hh, i * 128:(i + 1) * 128],
                                                    identb_s[:])
                                nc.vector.tensor_copy(
                                    hrow[:, i, hh * 128:(hh + 1) * 128], pt[:])
                        hv = h_own[gA * 128:(gA + 2) * 128, :].rearrange(
                            "(i p) d -> p i d", p=128)
                        nc.sync.dma_start(hv, hrow[:])

            if PHASES >= 5:
                allgather(h_own[:], h_full[:])

            if PHASES >= 6:
                agg2 = aggp.tile([128, NG * HID], bf16, tag="agg")
                with (
                    tc.tile_pool(name="e2_sb", bufs=3) as e2sb,
                    tc.tile_pool(name="e2_ps", bufs=4, space="PSUM") as e2ps,
                ):
                    edge_phase(2, HID, h_full, agg2, e2sb, e2ps)

            if PHASES < 7:
                with tc.tile_pool(name="dummy", bufs=1) as dup:
                    z = dup.tile([128, OUT_D], bf16)
                    nc.vector.memset(z[:], 0.0)
                    for gg in range(NG):
                        ovd = out_own[gg * 128:(gg + 1) * 128, :]
                        nc.sync.dma_start(ovd, z[:])
            if PHASES >= 7:
                # ---- dense layer 2 (row pairs) ------------------------------
                with (
                    tc.tile_pool(name="d2_sb", bufs=2) as dsb2,
                    tc.tile_pool(name="d2_pt", bufs=2, space="PSUM") as dpt2,
                    tc.tile_pool(name="d2_po", bufs=2, space="PSUM") as dpo2,
                ):
                    for pr in range(NG // 2):
                        gA = 2 * pr
                        aggT2 = dsb2.tile([128, 2, 256], bf16, tag="aggT2")
                        for i in range(2):
                            for k in range(2):
                                pt = dpt2.tile([128, 128], bf16, tag="ptb")
                                nc.tensor.transpose(
                                    pt[:],
                                    agg2[:, (gA + i) * HID + k * 128:
                                         (gA + i) * HID + (k + 1) * 128],
                                    identb_s[:])
                                nc.vector.tensor_copy(
                                    aggT2[:, k, i * 128:(i + 1) * 128], pt[:])
                        hTt2 = dsb2.tile([128, 2, 256], bf16, tag="hTt2")
                        for k in range(2):
                            nc.sync.dma_start(hTt2[:, k, :],
                                              hT_own[k][:, gA * 128:(gA + 2) * 128])
                        po = dpo2.tile([128, 256], f32, tag="po")
                        nc.tensor.matmul(po[:], W2rel_s[:, 0, :],
                                         aggT2[:, 0, :],
                                         start=True, stop=False)
                        nc.tensor.matmul(po[:], W2rel_s[:, 1, :],
                                         aggT2[:, 1, :],
                                         start=False, stop=False)
                        nc.tensor.matmul(po[:], W2root_s[:, 0, :],
                                         hTt2[:, 0, :],
                                         start=False, stop=False)
                        nc.tensor.matmul(po[:], W2root_s[:, 1, :],
                                         hTt2[:, 1, :],
                                         start=False, stop=True)
                        oT = dsb2.tile([128, 256], f32, tag="oT")
                        nc.vector.tensor_scalar_add(oT[:], po[:], b2_s[:, 0:1])
                        orow = dsb2.tile([128, 2, OUT_D], bf16, tag="orow")
                        for i in range(2):
                            pt = dpt2.tile([128, 128], f32, tag="ptd2")
                            nc.tensor.transpose(pt[:], oT[:, i * 128:(i + 1) * 128],
                                                ident_s[:])
                            nc.vector.tensor_copy(orow[:, i, :], pt[:])
                        ov = out_own[gA * 128:(gA + 2) * 128, :].rearrange(
                            "(i p) e -> p i e", p=128)
                        nc.sync.dma_start(ov, orow[:])

    nc.compile()
    return nc


_CACHE = {}


def kernel(**inputs):
    inputs = {k: np.asarray(v) for k, v in inputs.items()}
    plan, in_maps = _host_plan(**inputs)
    key = (plan["LTOT"], tuple(plan["chunk_q"]),
           os.environ.get("K_PHASES", "99"))
    if key not in _CACHE:
        _CACHE[key] = _build_nc(plan)
    nc = _CACHE[key]
    res = run_bass_kernel_spmd(nc, in_maps, list(range(NCORES)))
    out = np.empty((N, OUT_D), np.float32)
    for c in range(NCORES):
        out[c * OWN:(c + 1) * OWN] = res.results[c]["out_own"][:OWN].astype(np.float32)
    return out


# revision 13
# speedup vs baseline: 2171.9149x; 1.0165x over previous
"""Trainium2 Bass kernel for BipartiteSAGE-style 2-layer GraphConv.

Reference computation (N=120000 nodes, E=1e6 edges, EMB=128, HID=256, OUT=128):
    pol = relu(pol_features @ W_proj + b_proj) + state_emb[state_ids]   [100000,128]
    x   = concat([pol, emb_tick])                                        [N,128]
    agg = segment_sum(x[src]*w, dst);  h = relu(agg@W1_rel + b1 + x@W1root)
    agg2= segment_sum(h[src]*w, dst);  out = agg2@W2_rel + b2 + h@W2root

Distribution: 8 NeuronCores, edges sharded by DESTINATION range (each core owns
15000 nodes). Per-edge aggregation runs as PE matmuls: for each 128-edge block,
a [128 edges, 128 dst-slot] selection matrix A (edge weight at the edge's dst
slot) is multiplied against gathered source rows G (dma_gather from a
replicated node table), accumulating the weighted segment sum in PSUM. Source
rows are indexed within 4 "quadrant" windows (two cores' rows each, int16-
indexable); x and h are replicated between layers by one AllGather each.

All large host-built operands of the original baseline are gone:
  - A matrices are built ON DEVICE per 128-edge block from compact per-edge
    (slot, weight) vectors via one DVE tensor_scalar: A = (iota == slot) * w.
  - The gather index stream ships as [16, L/16] (the Q7 16-partition wrap) and
    is replicated to the 8 stripes on device; kept SBUF-resident.
  - The x-gather table (state_emb ++ emb_tick) is assembled on device: tick
    embeddings ship sharded (2500 rows/core, bf16) and are AllGathered.
  - Weights/features ship bf16 where the math is bf16 anyway.
  - Edge gathers round-robin over 4 SWDGE queues.
  - Output ships bf16 and is upcast on host.

Single SPMD program; all per-core differences are pure data (indices, slots,
weights, input slices). Block structure is core-invariant: per (quadrant, dst
group) edge runs are padded to the max block count over cores.
"""
import os
import sys
import numpy as np

for _p in ("/opt/trn_rl_repo",):
    if _p not in sys.path:
        sys.path.insert(0, _p)

import ml_dtypes  # noqa: E402
from concourse import bacc, tile, mybir  # noqa: E402
from concourse.bass_utils import run_bass_kernel_spmd  # noqa: E402

BF16 = ml_dtypes.bfloat16

# problem constants (hardcoded per harness contract)
N_POL, N_TICK = 100000, 20000
N = N_POL + N_TICK
E = 1000000
POL_FEAT, EMB, HID, OUT_D = 7, 128, 256, 128
N_STATES = 60

NCORES = 8
OWN = 15000            # real rows per core
NG = 118               # row groups of 128 per core
NSH = NG * 128         # padded rows per core (15104)
NFULL = NCORES * NSH   # padded global rows (120832)
QUAD = NFULL // 4      # gather window: two cores' rows (30208 < int16 max)
CH = int(os.environ.get("K_CH", "2048"))  # edges per gather chunk
BLK_PER_CH = CH // 128
NQUEUES = 4
GT_ROWS = 64 + N_TICK  # device-assembled gather table (state_emb ++ emb_tick)
TICK_SH = N_TICK // NCORES


def _wrap16(vals, width, dtype=np.int16):
    """[16, width] Q7 index layout: value j at [j%16, j//16]."""
    w = np.zeros((16, width), dtype)
    j = np.arange(len(vals))
    w[j % 16, j // 16] = vals
    return w


def _host_plan(pol_features, state_ids, edge_index, edge_weight,
               W_proj, b_proj, state_emb, emb_tick,
               W1_rel, b1_rel, W1_root, W2_rel, b2_rel, W2_root):
    src = edge_index[0].astype(np.int64)
    dst = edge_index[1].astype(np.int64)
    w = edge_weight.astype(np.float32)

    core = dst // OWN                      # owner of the destination
    doff = dst - core * OWN
    g = doff >> 7                          # local dst row-group
    slot = doff & 127                      # slot within group
    ps = (src // OWN) * NSH + (src % OWN)  # padded global source row
    q = ps // QUAD
    srel = (ps - q * QUAD).astype(np.int64)

    # counts per (core, quadrant, group)
    key = ((core * 4 + q) * NG + g).astype(np.int32)
    cnt = np.bincount(key, minlength=NCORES * 4 * NG).reshape(NCORES, 4, NG)
    B = -(-cnt // 128)                     # ceil
    B = B.max(axis=0)                      # [4, NG] uniform over cores

    # pad each quadrant's stream to a chunk multiple (extra blocks -> last group)
    for qq in range(4):
        lq = int(B[qq].sum()) * 128
        B[qq, NG - 1] += ((-lq) % CH) // 128
    S = B * 128                            # slots per (q, g)
    LTOT = int(S.sum())
    NB = LTOT // 128
    NCH = LTOT // CH

    # stream offsets per (q, g) in q-major order
    off = np.zeros((4, NG), np.int64)
    run = 0
    for qq in range(4):
        for gg in range(NG):
            off[qq, gg] = run
            run += S[qq, gg]

    blocks = []
    for qq in range(4):
        for gg in range(NG):
            nb = int(B[qq, gg])
            for i in range(nb):
                blocks.append((qq, gg, i == 0, i == nb - 1))
    assert len(blocks) == NB

    plan = dict(LTOT=LTOT, NB=NB, NCH=NCH, blocks=blocks,
                chunk_q=[blocks[ci * BLK_PER_CH][0] for ci in range(NCH)])

    # ---- per-core edge arrays (counting-sort into stream positions) ------
    order = np.argsort(key, kind="stable")
    ks = key[order]
    starts = np.r_[0, np.cumsum(np.bincount(ks, minlength=NCORES * 4 * NG))]
    rank = np.arange(E, dtype=np.int64) - starts[ks]
    off_flat = np.broadcast_to(off[None], (NCORES, 4, NG)).reshape(-1)
    jpos = off_flat[ks] + rank             # stream position within owner core
    srel_s = srel[order]
    slot_s = slot[order]
    w_s = w[order]
    core_bound = np.searchsorted(ks, np.arange(NCORES + 1) * (4 * NG))

    eidx_list, slot_list, wei_list = [], [], []
    for c in range(NCORES):
        lo, hi = core_bound[c], core_bound[c + 1]
        jj = jpos[lo:hi]
        ei = np.zeros((16, LTOT // 16), np.int16)
        ei[jj % 16, jj // 16] = srel_s[lo:hi]
        eidx_list.append(ei)
        sv = np.zeros((128, NB), BF16)
        sv[jj % 128, jj // 128] = slot_s[lo:hi]
        slot_list.append(sv)
        wv = np.zeros((128, NB), BF16)
        wv[jj % 128, jj // 128] = w_s[lo:hi]
        wei_list.append(wv)

    # ---- per-core node-feature arrays -----------------------------------
    polfT_list, sid_list, mask_list, tick_list = [], [], [], []
    for c in range(NCORES):
        rows = c * OWN + np.arange(NSH)
        is_real = np.arange(NSH) < OWN
        is_pol = is_real & (rows < N_POL)
        is_tick = is_real & (rows >= N_POL) & (rows < N)
        pT = np.zeros((8, NSH), BF16)
        pr = np.where(is_pol)[0]
        pT[:POL_FEAT, pr] = pol_features[rows[pr]].T.astype(BF16)
        pT[7, :] = 1.0
        polfT_list.append(pT)
        sid = np.zeros(NSH, np.int64)
        sid[pr] = state_ids[rows[pr]]
        tr = np.where(is_tick)[0]
        sid[tr] = 64 + (rows[tr] - N_POL)
        sid_list.append(_wrap16(sid, NSH // 16))
        mk = np.zeros((128, NG), np.float32)
        mk[np.arange(NSH) % 128, np.arange(NSH) // 128] = is_pol.astype(np.float32)
        mask_list.append(mk)
        tick_list.append(
            emb_tick[c * TICK_SH:(c + 1) * TICK_SH].astype(BF16))

    state_bf = np.zeros((64, EMB), BF16)
    state_bf[:N_STATES] = state_emb.astype(BF16)

    shared = dict(
        state_bf=state_bf,
        Wp=np.concatenate([W_proj.astype(BF16),
                           b_proj.astype(BF16)[None, :]], axis=0),
        W1rel=W1_rel.astype(BF16),
        W1root=W1_root.astype(BF16),
        b1c=b1_rel.astype(np.float32).reshape(2, 128).T.copy(),
        W2rel=W2_rel.astype(BF16).reshape(2, 128, 128),
        W2root=W2_root.astype(BF16).reshape(2, 128, 128),
        b2c=b2_rel.astype(np.float32).reshape(128, 1),
        ident=np.eye(128, dtype=np.float32),
    )
    in_maps = []
    for c in range(NCORES):
        m = dict(shared)
        m.update(eidx=eidx_list[c], slotv=slot_list[c], wei=wei_list[c],
                 polfT=polfT_list[c], sid=sid_list[c], mask=mask_list[c],
                 tick=tick_list[c])
        in_maps.append(m)
    return plan, in_maps


def _build_nc(plan):
    PHASES = int(os.environ.get("K_PHASES", "99"))
    dt = mybir.dt
    f32, bf16, i16, i32 = dt.float32, dt.bfloat16, dt.int16, dt.int32
    LTOT, NB, NCH = plan["LTOT"], plan["NB"], plan["NCH"]
    blocks, chunk_q = plan["blocks"], plan["chunk_q"]

    nc = bacc.Bacc("TRN2", target_bir_lowering=False, debug=False,
                   num_devices=NCORES, num_swdge_queues=NQUEUES)

    # inputs
    state_bf = nc.dram_tensor("state_bf", [64, EMB], bf16, kind="ExternalInput")
    tick = nc.dram_tensor("tick", [TICK_SH, EMB], bf16, kind="ExternalInput")
    Wp = nc.dram_tensor("Wp", [8, 128], bf16, kind="ExternalInput")
    W1rel = nc.dram_tensor("W1rel", [128, 256], bf16, kind="ExternalInput")
    W1root = nc.dram_tensor("W1root", [128, 256], bf16, kind="ExternalInput")
    b1c = nc.dram_tensor("b1c", [128, 2], f32, kind="ExternalInput")
    W2rel = nc.dram_tensor("W2rel", [2, 128, 128], bf16, kind="ExternalInput")
    W2root = nc.dram_tensor("W2root", [2, 128, 128], bf16, kind="ExternalInput")
    b2c = nc.dram_tensor("b2c", [128, 1], f32, kind="ExternalInput")
    ident = nc.dram_tensor("ident", [128, 128], f32, kind="ExternalInput")
    eidx = slotv = wei = None
    if PHASES >= 3:
        eidx = nc.dram_tensor("eidx", [16, LTOT // 16], i16, kind="ExternalInput")
        slotv = nc.dram_tensor("slotv", [128, NB], bf16, kind="ExternalInput")
        wei = nc.dram_tensor("wei", [128, NB], bf16, kind="ExternalInput")
    polfT = nc.dram_tensor("polfT", [8, NSH], bf16, kind="ExternalInput")
    sid = nc.dram_tensor("sid", [16, NSH // 16], i16, kind="ExternalInput")
    mask = nc.dram_tensor("mask", [128, NG], f32, kind="ExternalInput")

    out_own = nc.dram_tensor("out_own", [NSH, OUT_D], bf16, kind="ExternalOutput")

    # internals
    tick_i = nc.dram_tensor("tick_i", [TICK_SH, EMB], bf16)
    gt = nc.dram_tensor("gt", [GT_ROWS, EMB], bf16, addr_space="Shared")
    x_own = nc.dram_tensor("x_own", [NSH, EMB], bf16)
    xT_own = nc.dram_tensor("xT_own", [128, NSH], bf16)
    x_full = nc.dram_tensor("x_full", [NFULL, EMB], bf16, addr_space="Shared")
    h_own = nc.dram_tensor("h_own", [NSH, HID], bf16)
    hT_own = nc.dram_tensor("hT_own", [2, 128, NSH], bf16)
    h_full = nc.dram_tensor("h_full", [NFULL, HID], bf16, addr_space="Shared")

    rg = [list(range(NCORES))]

    def allgather(in_ap, out_ap):
        nc.gpsimd.collective_compute(
            "AllGather", mybir.AluOpType.bypass, replica_groups=rg,
            ins=[in_ap], outs=[out_ap])

    with tile.TileContext(nc) as tc:
        with (
            tc.tile_pool(name="const", bufs=1) as cp,
            tc.tile_pool(name="aggp", bufs=1) as aggp,
        ):
            # ---- constants -------------------------------------------------
            Wp_s = cp.tile([8, 128], bf16)
            nc.sync.dma_start(Wp_s[:], Wp[:])
            W1rel_s = cp.tile([128, 256], bf16)
            nc.sync.dma_start(W1rel_s[:], W1rel[:])
            W1root_s = cp.tile([128, 256], bf16)
            nc.sync.dma_start(W1root_s[:], W1root[:])
            b1_s = cp.tile([128, 2], f32)
            nc.sync.dma_start(b1_s[:], b1c[:])
            W2rel_s = cp.tile([128, 2, 128], bf16)
            W2root_s = cp.tile([128, 2, 128], bf16)
            for k in range(2):
                nc.sync.dma_start(W2rel_s[:, k, :], W2rel[k])
                nc.sync.dma_start(W2root_s[:, k, :], W2root[k])
            b2_s = cp.tile([128, 1], f32)
            nc.sync.dma_start(b2_s[:], b2c[:])
            ident_s = cp.tile([128, 128], f32)
            nc.sync.dma_start(ident_s[:], ident[:])
            identb_s = cp.tile([128, 128], bf16)
            nc.vector.tensor_copy(identb_s[:], ident_s[:])
            mask_s = cp.tile([128, NG], f32)
            nc.sync.dma_start(mask_s[:], mask[:])
            # free-dim iota 0..127, as f32 (for on-device A construction)
            io32 = cp.tile([128, 128], i32)
            nc.gpsimd.iota(io32[:], pattern=[[1, 128]], base=0,
                           channel_multiplier=0)
            ioF = cp.tile([128, 128], f32)
            nc.vector.tensor_copy(ioF[:], io32[:])

            # resident per-edge data: indices (replicated to 8 Q7 stripes),
            # dst slots and weights (f32 scalars for tensor_scalar)
            sidrep = cp.tile([128, NSH // 16], i16)
            for s in range(8):
                nc.sync.dma_start(sidrep[16 * s:16 * (s + 1), :], sid[:])
            if PHASES >= 3:
                eirep = cp.tile([128, LTOT // 16], i16)
                for s in range(8):
                    nc.sync.dma_start(eirep[16 * s:16 * (s + 1), :], eidx[:])
                slot_b = cp.tile([128, NB], bf16)
                nc.sync.dma_start(slot_b[:], slotv[:])
                slot_s = cp.tile([128, NB], f32)
                nc.vector.tensor_copy(slot_s[:], slot_b[:])
                wei_b = cp.tile([128, NB], bf16)
                nc.sync.dma_start(wei_b[:], wei[:])
                wei_s = cp.tile([128, NB], f32)
                nc.vector.tensor_copy(wei_s[:], wei_b[:])

            # ---- gather table: state_emb ++ AllGather(tick shards) ---------
            nc.sync.dma_start(gt[0:64, :], state_bf[:])
            nc.sync.dma_start(tick_i[:], tick[:])
            allgather(tick_i[:], gt[64:GT_ROWS, :])

            # ---- build x_own (+ xT_own) -----------------------------------
            with (
                tc.tile_pool(name="xb_sb", bufs=2) as xsb,
                tc.tile_pool(name="xb_ps", bufs=2, space="PSUM") as xps,
            ):
                done = 0
                while done < NSH:
                    nidx = min(CH, NSH - done)
                    nt = nidx // 128
                    xg = xsb.tile([128, BLK_PER_CH, EMB], bf16, tag="xg")
                    nc.gpsimd.dma_gather(
                        xg[:, :nt, :], gt[:],
                        sidrep[:, done // 16:(done + nidx) // 16],
                        nidx, nidx, EMB, single_packet=False)
                    polfc = xsb.tile([8, CH], bf16, tag="polfc")
                    nc.sync.dma_start(polfc[:, :nidx], polfT[:, done:done + nidx])
                    xrow = xsb.tile([128, BLK_PER_CH, EMB], bf16, tag="xrow")
                    for ti in range(nt):
                        t = done // 128 + ti
                        xf = xsb.tile([128, 128], f32, tag="xf")
                        px = xps.tile([128, 128], f32, tag="px")
                        nc.tensor.matmul(px[:], polfc[:, ti * 128:(ti + 1) * 128],
                                         Wp_s[:], start=True, stop=True)
                        nc.scalar.activation(xf[:], px[:],
                                             mybir.ActivationFunctionType.Relu)
                        nc.vector.tensor_scalar_mul(xf[:], xf[:], mask_s[:, t:t + 1])
                        nc.vector.tensor_add(xf[:], xf[:], xg[:, ti, :])
                        nc.vector.tensor_copy(xrow[:, ti, :], xf[:])
                        pt = xps.tile([128, 128], f32, tag="ptx")
                        nc.tensor.transpose(pt[:], xf[:], ident_s[:])
                        xT_t = xsb.tile([128, 128], bf16, tag="xTt")
                        nc.vector.tensor_copy(xT_t[:], pt[:])
                        nc.sync.dma_start(xT_own[:, t * 128:(t + 1) * 128], xT_t[:])
                    xv = x_own[done:done + nidx, :].rearrange("(t p) e -> p t e",
                                                              p=128)
                    nc.sync.dma_start(xv, xrow[:, :nt, :])
                    done += nidx

            if PHASES >= 2:
                allgather(x_own[:], x_full[:])

            # ---- edge phases ----------------------------------------------
            def edge_phase(layer, feat, src_full, agg, esb, eps):
                visited = set()
                cur = {}
                for ci in range(NCH):
                    qq = chunk_q[ci]
                    G = esb.tile([128, BLK_PER_CH, feat], bf16, tag="G")
                    nc.gpsimd.dma_gather(
                        G[:], src_full[qq * QUAD:(qq + 1) * QUAD, :],
                        eirep[:, ci * (CH // 16):(ci + 1) * (CH // 16)],
                        CH, CH, feat,
                        single_packet=False, queue_num=ci % NQUEUES)
                    for bi in range(BLK_PER_CH):
                        b = ci * BLK_PER_CH + bi
                        qb, gb, first, last = blocks[b]
                        Ab = esb.tile([128, 128], bf16, tag="Ab")
                        nc.vector.tensor_scalar(
                            Ab[:], ioF[:], slot_s[:, b:b + 1], wei_s[:, b:b + 1],
                            mybir.AluOpType.is_equal, mybir.AluOpType.mult)
                        if first:
                            cur[gb] = eps.tile([128, feat], f32, tag="ep",
                                               name=f"ep{layer}_{b}")
                        nc.tensor.matmul(cur[gb][:], Ab[:], G[:, bi, :],
                                         start=first, stop=last)
                        if last:
                            dstv = agg[:, gb * feat:(gb + 1) * feat]
                            if gb in visited:
                                nc.vector.tensor_add(dstv, dstv, cur[gb][:])
                            else:
                                nc.scalar.activation(
                                    dstv, cur[gb][:],
                                    mybir.ActivationFunctionType.Identity)
                                visited.add(gb)
                            del cur[gb]
                for gb in range(NG):
                    if gb not in visited:
                        nc.vector.memset(agg[:, gb * feat:(gb + 1) * feat], 0.0)

            if PHASES >= 3:
                agg1 = aggp.tile([128, NG * EMB], f32, tag="agg")
                with (
                    tc.tile_pool(name="e1_sb", bufs=4) as e1sb,
                    tc.tile_pool(name="e1_ps", bufs=4, space="PSUM") as e1ps,
                ):
                    edge_phase(1, EMB, x_full, agg1, e1sb, e1ps)

            if PHASES >= 4:
                # ---- dense layer 1 (row pairs) ------------------------------
                with (
                    tc.tile_pool(name="d1_sb", bufs=2) as dsb,
                    tc.tile_pool(name="d1_pt", bufs=2, space="PSUM") as dpt,
                    tc.tile_pool(name="d1_ph", bufs=2, space="PSUM") as dph,
                ):
                    for pr in range(NG // 2):
                        gA = 2 * pr
                        aggT = dsb.tile([128, 256], bf16, tag="aggT")
                        for i in range(2):
                            pt = dpt.tile([128, 128], f32, tag="ptd")
                            nc.tensor.transpose(
                                pt[:], agg1[:, (gA + i) * 128:(gA + i + 1) * 128],
                                ident_s[:])
                            nc.vector.tensor_copy(aggT[:, i * 128:(i + 1) * 128], pt[:])
                        xTt = dsb.tile([128, 256], bf16, tag="xTt2")
                        nc.sync.dma_start(xTt[:], xT_own[:, gA * 128:(gA + 2) * 128])
                        hTt = dsb.tile([128, 2, 256], bf16, tag="hTt")
                        for hh in range(2):
                            ph = dph.tile([128, 256], f32, tag="ph")
                            nc.tensor.matmul(ph[:],
                                             W1rel_s[:, hh * 128:(hh + 1) * 128],
                                             aggT[:], start=True, stop=False)
                            nc.tensor.matmul(ph[:],
                                             W1root_s[:, hh * 128:(hh + 1) * 128],
                                             xTt[:], start=False, stop=True)
                            nc.scalar.activation(hTt[:, hh, :], ph[:],
                                                 mybir.ActivationFunctionType.Relu,
                                                 bias=b1_s[:, hh:hh + 1])
                            nc.sync.dma_start(hT_own[hh][:, gA * 128:(gA + 2) * 128],
                                              hTt[:, hh, :])
                        hrow = dsb.tile([128, 2, HID], bf16, tag="hrow")
                        for i in range(2):
                            for hh in range(2):
                                pt = dpt.tile([128, 128], bf16, tag="ptdb")
                                nc.tensor.transpose(pt[:],
                                                    hTt[:, hh, i * 128:(i + 1) * 128],
                                                    identb_s[:])
                                nc.vector.tensor_copy(
                                    hrow[:, i, hh * 128:(hh + 1) * 128], pt[:])
                        hv = h_own[gA * 128:(gA + 2) * 128, :].rearrange(
                            "(i p) d -> p i d", p=128)
                        nc.sync.dma_start(hv, hrow[:])

            if PHASES >= 5:
                allgather(h_own[:], h_full[:])

            if PHASES >= 6:
                agg2 = aggp.tile([128, NG * HID], bf16, tag="agg")
                with (
                    tc.tile_pool(name="e2_sb", bufs=3) as e2sb,
                    tc.tile_pool(name="e2_ps", bufs=4, space="PSUM") as e2ps,
                ):
                    edge_phase(2, HID, h_full, agg2, e2sb, e2ps)

            if PHASES < 7:
                with tc.tile_pool(name="dummy", bufs=1) as dup:
                    z = dup.tile([128, OUT_D], bf16)
                    nc.vector.memset(z[:], 0.0)
                    for gg in range(NG):
                        ovd = out_own[gg * 128:(gg + 1) * 128, :]
                        nc.sync.dma_start(ovd, z[:])
            if PHASES >= 7:
                # ---- dense layer 2 (row pairs) ------------------------------
                with (
                    tc.tile_pool(name="d2_sb", bufs=2) as dsb2,
                    tc.tile_pool(name="d2_pt", bufs=2, space="PSUM") as dpt2,
                    tc.tile_pool(name="d2_po", bufs=2, space="PSUM") as dpo2,
                ):
                    for pr in range(NG // 2):
                        gA = 2 * pr
                        aggT2 = dsb2.tile([128, 2, 256], bf16, tag="aggT2")
                        for i in range(2):
                            for k in range(2):
                                pt = dpt2.tile([128, 128], bf16, tag="ptb")
                                nc.tensor.transpose(
                                    pt[:],
                                    agg2[:, (gA + i) * HID + k * 128:
                                         (gA + i) * HID + (k + 1) * 128],
                                    identb_s[:])
                                nc.vector.tensor_copy(
                                    aggT2[:, k, i * 128:(i + 1) * 128], pt[:])
                        hTt2 = dsb2.tile([128, 2, 256], bf16, tag="hTt2")
                        for k in range(2):
                            nc.sync.dma_start(hTt2[:, k, :],
                                              hT_own[k][:, gA * 128:(gA + 2) * 128])
                        po = dpo2.tile([128, 256], f32, tag="po")
                        nc.tensor.matmul(po[:], W2rel_s[:, 0, :],
                                         aggT2[:, 0, :],
                                         start=True, stop=False)
                        nc.tensor.matmul(po[:], W2rel_s[:, 1, :],
                                         aggT2[:, 1, :],
                                         start=False, stop=False)
                        nc.tensor.matmul(po[:], W2root_s[:, 0, :],
                                         hTt2[:, 0, :],
                                         start=False, stop=False)
                        nc.tensor.matmul(po[:], W2root_s[:, 1, :],
                                         hTt2[:, 1, :],
                                         start=False, stop=True)
                        oT = dsb2.tile([128, 256], f32, tag="oT")
                        nc.vector.tensor_scalar_add(oT[:], po[:], b2_s[:, 0:1])
                        orow = dsb2.tile([128, 2, OUT_D], bf16, tag="orow")
                        for i in range(2):
                            pt = dpt2.tile([128, 128], f32, tag="ptd2")
                            nc.tensor.transpose(pt[:], oT[:, i * 128:(i + 1) * 128],
                                                ident_s[:])
                            nc.vector.tensor_copy(orow[:, i, :], pt[:])
                        ov = out_own[gA * 128:(gA + 2) * 128, :].rearrange(
                            "(i p) e -> p i e", p=128)
                        nc.sync.dma_start(ov, orow[:])

    nc.compile()
    return nc


_CACHE = {}


def kernel(**inputs):
    inputs = {k: np.asarray(v) for k, v in inputs.items()}
    plan, in_maps = _host_plan(**inputs)
    key = (plan["LTOT"], tuple(plan["chunk_q"]),
           os.environ.get("K_PHASES", "99"))
    if key not in _CACHE:
        _CACHE[key] = _build_nc(plan)
    nc = _CACHE[key]
    res = run_bass_kernel_spmd(nc, in_maps, list(range(NCORES)))
    out = np.empty((N, OUT_D), np.float32)
    for c in range(NCORES):
        out[c * OWN:(c + 1) * OWN] = res.results[c]["out_own"][:OWN].astype(np.float32)
    return out
